# revision 10
# baseline (speedup 1.0000x reference)
"""Trainium2 Bass kernel for a cross-attention nn.Module.

Computes, for inputs (all fp32):
    q      = output @ attn_weight.T              [B,T,H]
    attn   = softmax(q @ context.T, axis=-1)     [B,T,S]
    mix    = attn @ context                      [B,T,H]
    out    = tanh(concat(mix, q) @ linear_out_w.T + linear_out_b)
Returns (out, attn).

Sharding: data-parallel over batch, 2 batches per NeuronCore x 8 cores,
no collectives.  Inside each core:
  - q and scores matmuls run in true fp32 (softmax is extremely sensitive:
    near-one-hot rows; bf16/tf32 scores give absmax errors ~0.02-0.16).
  - mix and linear_out matmuls run as float32r (full PE rate at N>=256).
  - layouts: contraction dims must sit on SBUF partitions, so context is
    PE-transposed once per batch (cT), attn is transposed per tile (attnT),
    mix is computed in natural [t,h] layout then transposed for linear_out.
"""

import os
import sys
import types
import numpy as np
from contextlib import ExitStack

import concourse.bass as bass
import concourse.mybir as mybir
import concourse.tile as tile
from concourse import bass_utils
from concourse.masks import make_identity


def _ensure_ntff_hook():
    """This deployment's antenv package lacks axon_hooks, which
    run_bass_kernel_spmd(trace=True) imports under axon.  Register a shim that
    drives NTFF profiling via ctypes into libaxon_pjrt.so (same contract as
    trn_agent_boot._ntff_profile_via_ctypes)."""
    try:
        from antenv.axon_hooks import get_axon_ntff_profile_hook  # noqa: F401
        return
    except ImportError:
        pass
    import contextlib
    import ctypes

    so_path = "/opt/axon/libaxon_pjrt.so"
    hook = None
    if os.path.exists(so_path):
        lib = ctypes.CDLL(so_path)
        if hasattr(lib, "axon_start_nrt_profile"):
            lib.axon_start_nrt_profile.argtypes = [
                ctypes.POINTER(ctypes.c_int64), ctypes.c_size_t]
            lib.axon_start_nrt_profile.restype = ctypes.c_int64
            lib.axon_stop_nrt_profile.argtypes = [ctypes.c_char_p]
            lib.axon_stop_nrt_profile.restype = ctypes.c_int64

            @contextlib.contextmanager
            def _hook(output_dir, device_ids):
                import jax
                jax.devices()
                if device_ids:
                    ids = (ctypes.c_int64 * len(device_ids))(*device_ids)
                    rc = lib.axon_start_nrt_profile(ids, len(device_ids))
                else:
                    rc = lib.axon_start_nrt_profile(None, 0)
                if rc != 0:
                    raise RuntimeError(f"axon_start_nrt_profile rc={rc}")
                try:
                    yield
                finally:
                    n = lib.axon_stop_nrt_profile(str(output_dir).encode())
                    print(f"ntff profile: {n} file(s) -> {output_dir}")

            hook = _hook

    mod = types.ModuleType("antenv.axon_hooks")
    mod._hook = hook
    mod.get_axon_ntff_profile_hook = lambda: mod._hook
    mod.set_axon_ntff_profile_hook = lambda h: setattr(mod, "_hook", h)
    sys.modules["antenv.axon_hooks"] = mod
    import antenv
    antenv.axon_hooks = mod


_ensure_ntff_hook()

F32 = mybir.dt.float32
F32R = mybir.dt.float32r
AF = mybir.ActivationFunctionType
ALU = mybir.AluOpType
AX = mybir.AxisListType

B, T, S, H = 16, 1024, 2048, 1024
NCORES = 8
BL = B // NCORES            # batches per core
T_CHK = 256                 # t rows per pipeline chunk
N_CHUNKS = T // T_CHK       # 4 per batch
TSUBS = T_CHK // 128        # 2 t-tiles per chunk
HT = H // 128               # 8 feature tiles
ST = S // 128               # 16 context-position tiles
KT2 = 2 * H // 128          # 16 contraction tiles for linear_out


def _emit(ctx: ExitStack, tc: "tile.TileContext", io: dict):
    nc = tc.nc
    a_out, a_attn = io["out"], io["attn"]
    a_output, a_context = io["output"], io["context"]
    a_w, a_w2, a_b2 = io["attn_weight"], io["linear_out_w"], io["linear_out_b"]

    const = ctx.enter_context(tc.tile_pool(name="const", bufs=1))
    ct_pool = ctx.enter_context(tc.tile_pool(name="ct", bufs=1))
    loadc = ctx.enter_context(tc.tile_pool(name="loadc", bufs=2))
    loadout = ctx.enter_context(tc.tile_pool(name="loadout", bufs=2))
    outT_pool = ctx.enter_context(tc.tile_pool(name="outT", bufs=1))
    qT_pool = ctx.enter_context(tc.tile_pool(name="qT", bufs=1))
    attnT_pool = ctx.enter_context(tc.tile_pool(name="attnT", bufs=1))
    mixnat_pool = ctx.enter_context(tc.tile_pool(name="mixnat", bufs=1))
    mixT_pool = ctx.enter_context(tc.tile_pool(name="mixT", bufs=1))
    astage = ctx.enter_context(tc.tile_pool(name="astage", bufs=2))
    ostage = ctx.enter_context(tc.tile_pool(name="ostage", bufs=2))
    w2s_pool = ctx.enter_context(tc.tile_pool(name="w2s", bufs=3))
    small = ctx.enter_context(tc.tile_pool(name="small", bufs=8))
    dram = ctx.enter_context(tc.tile_pool(name="dram", bufs=1, space="DRAM"))
    # PSUM: tag "mm" [128,1024]f32 = 2 banks x3 bufs; tag "tr" [128,4,128] = 1 bank x2
    ps_mm = ctx.enter_context(tc.tile_pool(name="ps_mm", bufs=3, space="PSUM"))
    ps_tr = ctx.enter_context(tc.tile_pool(name="ps_tr", bufs=2, space="PSUM"))

    def transpose_pack(src_ap, n, dst_ap, ident):
        """Transpose n [128,128] column blocks of src_ap into dst_ap [128,n,128]."""
        assert n <= 4
        pst = ps_tr.tile([128, 4, 128], F32, tag="tr")
        for i in range(n):
            nc.tensor.transpose(pst[:, i, :], src_ap[:, i * 128:(i + 1) * 128], ident)
        nc.vector.tensor_copy(dst_ap, pst[:, :n, :])

    # ---------------- phase 0: constants ----------------
    ident = const.tile([128, 128], F32)
    make_identity(nc, ident)

    b2bc = const.tile([128, H], F32)
    nc.gpsimd.dma_start(out=b2bc, in_=a_b2.partition_broadcast(128))

    # WT[p, kt, o] = attn_weight[o, kt*128+p]  (h on partitions)
    WT = const.tile([128, HT, H], F32)
    for og in range(HT):
        wl = astage.tile([128, 2 * H], F32, tag="astage")
        nc.sync.dma_start(out=wl[:, :H], in_=a_w[og * 128:(og + 1) * 128, :])
        for hg in range(2):
            transpose_pack(wl[:, hg * 512:(hg + 1) * 512], 4,
                           WT[:, hg * 4:(hg + 1) * 4, og * 128:(og + 1) * 128], ident)

    # W2T in DRAM scratch: w2t[k, o] = linear_out_w[o, k]
    w2t = dram.tile([2 * H, H], F32R)
    for og in range(HT):
        w2l = astage.tile([128, 2 * H], F32, tag="astage")
        nc.sync.dma_start(out=w2l, in_=a_w2[og * 128:(og + 1) * 128, :])
        for kg in range(4):
            stg = mixT_pool.tile([128, 4, 128], F32R, tag="w2stg")
            pst = ps_tr.tile([128, 4, 128], F32, tag="tr")
            for ki in range(4):
                kt = kg * 4 + ki
                nc.tensor.transpose(pst[:, ki, :], w2l[:, kt * 128:(kt + 1) * 128], ident)
            nc.vector.tensor_copy(stg, pst)
            dst = w2t[kg * 512:(kg + 1) * 512, og * 128:(og + 1) * 128]
            nc.sync.dma_start(out=dst.rearrange("(a p) o -> p a o", p=128), in_=stg)

    # ---------------- per batch ----------------
    for b in range(BL):
        # cT[p, kt, s] = context[b, s, kt*128+p]  (h on partitions, s free)
        cT = ct_pool.tile([128, HT, S], F32)
        for st in range(ST):
            cl = loadc.tile([128, H], F32, tag="cload")
            nc.sync.dma_start(out=cl, in_=a_context[b, st * 128:(st + 1) * 128, :])
            for hg in range(2):
                transpose_pack(cl[:, hg * 512:(hg + 1) * 512], 4,
                               cT[:, hg * 4:(hg + 1) * 4, st * 128:(st + 1) * 128], ident)

        for ck in range(N_CHUNKS):
            t0 = ck * T_CHK

            # --- A: outputT[p, kt, t] = output[b, t0+t, kt*128+p] ---
            outT = outT_pool.tile([128, HT, T_CHK], F32)
            for tsub in range(TSUBS):
                ld = loadout.tile([128, H], F32, tag="oload")
                nc.sync.dma_start(
                    out=ld, in_=a_output[b, t0 + tsub * 128:t0 + (tsub + 1) * 128, :])
                for hg in range(2):
                    transpose_pack(ld[:, hg * 512:(hg + 1) * 512], 4,
                                   outT[:, hg * 4:(hg + 1) * 4,
                                        tsub * 128:(tsub + 1) * 128], ident)

            # --- B: qT[p, ot, t] = q[t0+t, ot*128+p], fp32 ---
            qT = qT_pool.tile([128, HT, T_CHK], F32)
            for og in range(4):
                psq = ps_mm.tile([128, 1024], F32, tag="mm")
                for oi in range(2):
                    ot = og * 2 + oi
                    for kt in range(HT):
                        nc.tensor.matmul(
                            psq[:, oi * 512:oi * 512 + T_CHK],
                            WT[:, kt, ot * 128:(ot + 1) * 128],
                            outT[:, kt, :],
                            start=(kt == 0), stop=(kt == HT - 1))
                nc.vector.tensor_copy(
                    qT[:, og * 2:og * 2 + 2, :],
                    psq.rearrange("p (a c) -> p a c", a=2)[:, :, :T_CHK])

            # --- C: scores + softmax + attn output + attnT, per t-tile ---
            attnT = attnT_pool.tile([128, ST, T_CHK], F32R)
            for tsub in range(TSUBS):
                r0 = t0 + tsub * 128
                ps_a = ps_mm.tile([128, 1024], F32, tag="mm")
                ps_b = ps_mm.tile([128, 1024], F32, tag="mm")
                for kt in range(HT):
                    qtile = qT[:, kt, tsub * 128:(tsub + 1) * 128]
                    for sc in range(4):
                        tgt = ps_a if sc < 2 else ps_b
                        nc.tensor.matmul(
                            tgt[:, (sc % 2) * 512:(sc % 2 + 1) * 512],
                            qtile,
                            cT[:, kt, sc * 512:(sc + 1) * 512],
                            start=(kt == 0), stop=(kt == HT - 1))
                nm_a = small.tile([128, 1], F32, tag="sm")
                nm_b = small.tile([128, 1], F32, tag="sm")
                nc.vector.tensor_reduce(nm_a, ps_a, axis=AX.X, op=ALU.max, negate=True)
                nc.vector.tensor_reduce(nm_b, ps_b, axis=AX.X, op=ALU.max, negate=True)
                nm = small.tile([128, 1], F32, tag="sm")
                nc.vector.tensor_tensor(nm, nm_a, nm_b, op=ALU.min)
                ast = astage.tile([128, S], F32, tag="astage")
                l_a = small.tile([128, 1], F32, tag="sm")
                l_b = small.tile([128, 1], F32, tag="sm")
                nc.scalar.activation(ast[:, :1024], ps_a, AF.Exp,
                                     bias=nm, scale=1.0, accum_out=l_a)
                nc.scalar.activation(ast[:, 1024:], ps_b, AF.Exp,
                                     bias=nm, scale=1.0, accum_out=l_b)
                lsum = small.tile([128, 1], F32, tag="sm")
                nc.vector.tensor_tensor(lsum, l_a, l_b, op=ALU.add)
                rinv = small.tile([128, 1], F32, tag="sm")
                nc.vector.reciprocal(rinv, lsum)
                nc.vector.tensor_scalar_mul(ast, ast, rinv)
                nc.sync.dma_start(out=a_attn[b, r0:r0 + 128, :], in_=ast)
                for sg in range(4):
                    transpose_pack(ast[:, sg * 512:(sg + 1) * 512], 4,
                                   attnT[:, sg * 4:(sg + 1) * 4,
                                         tsub * 128:(tsub + 1) * 128], ident)

            # --- D: mix natural [t,h] (f32r), then transpose to mixT[k,t] ---
            psm = [ps_mm.tile([128, 1024], F32, tag="mm", name=f"psm{i}")
                   for i in range(TSUBS)]
            for st in range(ST):
                cm = loadc.tile([128, H], F32R, tag="cload")
                nc.gpsimd.dma_start(out=cm, in_=a_context[b, st * 128:(st + 1) * 128, :])
                for tsub in range(TSUBS):
                    lhsT = attnT[:, st, tsub * 128:(tsub + 1) * 128]
                    for nchk in range(2):
                        nc.tensor.matmul(
                            psm[tsub][:, nchk * 512:(nchk + 1) * 512],
                            lhsT,
                            cm[:, nchk * 512:(nchk + 1) * 512],
                            start=(st == 0), stop=(st == ST - 1))
            combT = mixT_pool.tile([128, KT2, T_CHK], F32R, tag="combT")
            for tsub in range(TSUBS):
                mn = mixnat_pool.tile([128, H], F32, tag="mixnat")
                nc.vector.tensor_copy(mn, psm[tsub])
                for hg in range(2):
                    transpose_pack(mn[:, hg * 512:(hg + 1) * 512], 4,
                                   combT[:, hg * 4:(hg + 1) * 4,
                                         tsub * 128:(tsub + 1) * 128], ident)

            # --- E: linear_out (f32r) + bias + tanh ---
            # q half of combT: rounded copy of qT (f32 -> f32r)
            nc.vector.tensor_copy(combT[:, HT:, :], qT)
            pso = [ps_mm.tile([128, 1024], F32, tag="mm", name=f"pso{i}")
                   for i in range(TSUBS)]
            for kt in range(KT2):
                w2s = w2s_pool.tile([128, H], F32R, tag="w2s")
                nc.sync.dma_start(out=w2s, in_=w2t[kt * 128:(kt + 1) * 128, :])
                for tsub in range(TSUBS):
                    lhsT = combT[:, kt, tsub * 128:(tsub + 1) * 128]
                    for nchk in range(2):
                        nc.tensor.matmul(
                            pso[tsub][:, nchk * 512:(nchk + 1) * 512],
                            lhsT,
                            w2s[:, nchk * 512:(nchk + 1) * 512],
                            start=(kt == 0), stop=(kt == KT2 - 1))
            for tsub in range(TSUBS):
                nc.vector.tensor_tensor(pso[tsub], pso[tsub], b2bc, op=ALU.add)
                ost = ostage.tile([128, H], F32, tag="ostage")
                nc.scalar.activation(ost, pso[tsub], AF.Tanh)
                r0 = t0 + tsub * 128
                nc.sync.dma_start(out=a_out[b, r0:r0 + 128, :], in_=ost)


def _split_sync_waits(nc):
    """This walrus/ISA build accepts at most ONE sync-wait command per
    instruction, but Tile's sem-assigner can attach several (phase-first
    instructions, kernel-tail drain).  Split: keep the last wait on the
    instruction, hoist the rest onto same-engine NoOps inserted just before."""
    n_split = 0
    for fn in nc.m.functions:
        for blk in fn.blocks:
            insts = blk.instructions
            out = []
            changed = False
            for inst in insts:
                si = inst.sync_info
                waits = list(si.on_wait) if (si and si.on_wait) else []
                if len(waits) > 1:
                    for w in waits[:-1]:
                        nop = mybir.InstNoOp(
                            name=f"waitsplit-{nc.next_id()}",
                            engine=inst.engine,
                            sync_info=mybir.SyncInfo(on_wait=[w], on_update=[]),
                        )
                        out.append(nop)
                    inst.sync_info = mybir.SyncInfo(
                        on_wait=[waits[-1]], on_update=list(si.on_update or []))
                    n_split += 1
                    changed = True
                out.append(inst)
            if changed:
                blk.instructions = out
    return n_split


_CACHED_NC = {}


def _build(split_waits=True):
    if split_waits in _CACHED_NC:
        return _CACHED_NC[split_waits]
    nc = bass.Bass("TRN2", target_bir_lowering=False, debug=False)
    io = {
        "output": nc.dram_tensor("output", [BL, T, H], F32, kind="ExternalInput").ap(),
        "context": nc.dram_tensor("context", [BL, S, H], F32, kind="ExternalInput").ap(),
        "attn_weight": nc.dram_tensor("attn_weight", [H, H], F32, kind="ExternalInput").ap(),
        "linear_out_w": nc.dram_tensor("linear_out_w", [H, 2 * H], F32, kind="ExternalInput").ap(),
        "linear_out_b": nc.dram_tensor("linear_out_b", [H], F32, kind="ExternalInput").ap(),
        "out": nc.dram_tensor("out", [BL, T, H], F32, kind="ExternalOutput").ap(),
        "attn": nc.dram_tensor("attn", [BL, T, S], F32, kind="ExternalOutput").ap(),
    }
    with tile.TileContext(nc) as tc:
        with ExitStack() as ctx:
            _emit(ctx, tc, io)
    if split_waits:
        # CoreSim can't execute the bare NoOps; only split for the HW path.
        _split_sync_waits(nc)
    _CACHED_NC[split_waits] = nc
    return nc


def make_in_maps(inputs):
    in_maps = []
    for c in range(NCORES):
        b0 = c * BL
        in_maps.append({
            "output": np.ascontiguousarray(inputs["output"][b0:b0 + BL], dtype=np.float32),
            "context": np.ascontiguousarray(inputs["context"][b0:b0 + BL], dtype=np.float32),
            "attn_weight": np.ascontiguousarray(inputs["attn_weight"], dtype=np.float32),
            "linear_out_w": np.ascontiguousarray(inputs["linear_out_w"], dtype=np.float32),
            "linear_out_b": np.ascontiguousarray(inputs["linear_out_b"], dtype=np.float32),
        })
    return in_maps


LAST_RESULT = None


def kernel(**inputs):
    global LAST_RESULT
    nc = _build()
    in_maps = make_in_maps(inputs)
    trace = os.environ.get("KERNEL_TRACE", "0") == "1"
    res = bass_utils.run_bass_kernel_spmd(
        nc, in_maps, core_ids=list(range(NCORES)), trace=trace)
    LAST_RESULT = res
    out = np.concatenate([r["out"] for r in res.results], axis=0)
    attn = np.concatenate([r["attn"] for r in res.results], axis=0)
    return out, attn


if __name__ == "__main__":
    rng = np.random.default_rng(0)
    inputs = {
        "output": rng.standard_normal((B, T, H), dtype=np.float32),
        "context": rng.standard_normal((B, S, H), dtype=np.float32),
        "attn_weight": (rng.standard_normal((H, H)) / np.sqrt(H)).astype(np.float32),
        "linear_out_w": (rng.standard_normal((H, 2 * H)) / np.sqrt(2 * H)).astype(np.float32),
        "linear_out_b": (rng.standard_normal(H) * 0.01).astype(np.float32),
    }
    out, attn = kernel(**inputs)
    print("out", out.shape, "attn", attn.shape)


# revision 12
# speedup vs baseline: 1.0065x; 1.0065x over previous
"""Trainium2 Bass kernel for a cross-attention nn.Module.

Computes, for inputs (all fp32):
    q      = output @ attn_weight.T              [B,T,H]
    attn   = softmax(q @ context.T, axis=-1)     [B,T,S]
    mix    = attn @ context                      [B,T,H]
    out    = tanh(concat(mix, q) @ linear_out_w.T + linear_out_b)
Returns (out, attn).

Sharding: data-parallel over batch, 2 batches per NeuronCore x 8 cores,
no collectives.  Inside each core:
  - q and scores matmuls run in true fp32 (softmax is extremely sensitive:
    near-one-hot rows; bf16/tf32 scores give absmax errors ~0.02-0.16).
  - mix and linear_out matmuls run as float32r (full PE rate at N>=256).
  - layouts: contraction dims must sit on SBUF partitions, so context is
    PE-transposed once per batch (cT), attn is transposed per tile (attnT),
    mix is computed in natural [t,h] layout then transposed for linear_out.
"""

import os
import sys
import types
import numpy as np
from contextlib import ExitStack

import concourse.bass as bass
import concourse.mybir as mybir
import concourse.tile as tile
from concourse import bass_utils
from concourse.masks import make_identity


def _ensure_ntff_hook():
    """This deployment's antenv package lacks axon_hooks, which
    run_bass_kernel_spmd(trace=True) imports under axon.  Register a shim that
    drives NTFF profiling via ctypes into libaxon_pjrt.so (same contract as
    trn_agent_boot._ntff_profile_via_ctypes)."""
    try:
        from antenv.axon_hooks import get_axon_ntff_profile_hook  # noqa: F401
        return
    except ImportError:
        pass
    import contextlib
    import ctypes

    so_path = "/opt/axon/libaxon_pjrt.so"
    hook = None
    if os.path.exists(so_path):
        lib = ctypes.CDLL(so_path)
        if hasattr(lib, "axon_start_nrt_profile"):
            lib.axon_start_nrt_profile.argtypes = [
                ctypes.POINTER(ctypes.c_int64), ctypes.c_size_t]
            lib.axon_start_nrt_profile.restype = ctypes.c_int64
            lib.axon_stop_nrt_profile.argtypes = [ctypes.c_char_p]
            lib.axon_stop_nrt_profile.restype = ctypes.c_int64

            @contextlib.contextmanager
            def _hook(output_dir, device_ids):
                import jax
                jax.devices()
                if device_ids:
                    ids = (ctypes.c_int64 * len(device_ids))(*device_ids)
                    rc = lib.axon_start_nrt_profile(ids, len(device_ids))
                else:
                    rc = lib.axon_start_nrt_profile(None, 0)
                if rc != 0:
                    raise RuntimeError(f"axon_start_nrt_profile rc={rc}")
                try:
                    yield
                finally:
                    n = lib.axon_stop_nrt_profile(str(output_dir).encode())
                    print(f"ntff profile: {n} file(s) -> {output_dir}")

            hook = _hook

    mod = types.ModuleType("antenv.axon_hooks")
    mod._hook = hook
    mod.get_axon_ntff_profile_hook = lambda: mod._hook
    mod.set_axon_ntff_profile_hook = lambda h: setattr(mod, "_hook", h)
    sys.modules["antenv.axon_hooks"] = mod
    import antenv
    antenv.axon_hooks = mod


_ensure_ntff_hook()

F32 = mybir.dt.float32
F32R = mybir.dt.float32r
BF16 = mybir.dt.bfloat16
AF = mybir.ActivationFunctionType
ALU = mybir.AluOpType
AX = mybir.AxisListType

B, T, S, H = 16, 1024, 2048, 1024
NCORES = 8
BL = B // NCORES            # batches per core
T_CHK = 256                 # t rows per pipeline chunk
N_CHUNKS = T // T_CHK       # 4 per batch
TSUBS = T_CHK // 128        # 2 t-tiles per chunk
HT = H // 128               # 8 feature tiles
ST = S // 128               # 16 context-position tiles
KT2 = 2 * H // 128          # 16 contraction tiles for linear_out


def _emit(ctx: ExitStack, tc: "tile.TileContext", io: dict):
    nc = tc.nc
    a_out, a_attn = io["out"], io["attn"]
    a_output, a_context = io["output"], io["context"]
    a_w, a_w2, a_b2 = io["attn_weight"], io["linear_out_w"], io["linear_out_b"]

    const = ctx.enter_context(tc.tile_pool(name="const", bufs=1))
    ct_pool = ctx.enter_context(tc.tile_pool(name="ct", bufs=1))
    loadc = ctx.enter_context(tc.tile_pool(name="loadc", bufs=2))
    loadout = ctx.enter_context(tc.tile_pool(name="loadout", bufs=2))
    outT_pool = ctx.enter_context(tc.tile_pool(name="outT", bufs=1))
    qT_pool = ctx.enter_context(tc.tile_pool(name="qT", bufs=1))
    attnT_pool = ctx.enter_context(tc.tile_pool(name="attnT", bufs=1))
    mixnat_pool = ctx.enter_context(tc.tile_pool(name="mixnat", bufs=1))
    mixT_pool = ctx.enter_context(tc.tile_pool(name="mixT", bufs=1))
    astage = ctx.enter_context(tc.tile_pool(name="astage", bufs=2))
    ostage = ctx.enter_context(tc.tile_pool(name="ostage", bufs=1))
    w2s_pool = ctx.enter_context(tc.tile_pool(name="w2s", bufs=2))
    small = ctx.enter_context(tc.tile_pool(name="small", bufs=8))
    dram = ctx.enter_context(tc.tile_pool(name="dram", bufs=1, space="DRAM"))
    # PSUM: tag "mm" [128,1024]f32 = 2 banks x3 bufs; tag "tr" [128,4,128] = 1 bank x2
    ps_mm = ctx.enter_context(tc.tile_pool(name="ps_mm", bufs=3, space="PSUM"))
    ps_tr = ctx.enter_context(tc.tile_pool(name="ps_tr", bufs=2, space="PSUM"))

    def transpose_to_psum(src_ap, n, ident):
        """Transpose n [128,128] column blocks of src_ap into a psum pack."""
        assert n <= 4
        pst = ps_tr.tile([128, 4, 128], F32, tag="tr")
        for i in range(n):
            nc.tensor.transpose(pst[:, i, :], src_ap[:, i * 128:(i + 1) * 128], ident)
        return pst

    def transpose_pack(src_ap, n, dst_ap, ident):
        pst = transpose_to_psum(src_ap, n, ident)
        nc.vector.tensor_copy(dst_ap, pst[:, :n, :])

    def transpose_split(src_ap, n, dst_hi, dst_lo, ident):
        """Transpose then split into bf16 hi (ACT cast) + lo (DVE subtract)."""
        pst = transpose_to_psum(src_ap, n, ident)
        nc.scalar.copy(dst_hi, pst[:, :n, :])
        nc.vector.tensor_tensor(dst_lo, pst[:, :n, :], dst_hi, op=ALU.subtract)

    # ---------------- phase 0: constants ----------------
    ident = const.tile([128, 128], F32)
    make_identity(nc, ident)

    b2bc = const.tile([128, H], F32)
    nc.gpsimd.dma_start(out=b2bc, in_=a_b2.partition_broadcast(128))

    # WTh/WTl[p, kt, o]: bf16 hi/lo split of attn_weight[o, kt*128+p]
    WTh = const.tile([128, HT, H], BF16)
    WTl = const.tile([128, HT, H], BF16)
    for og in range(HT):
        wl = astage.tile([128, 2 * H], F32, tag="astage")
        nc.sync.dma_start(out=wl[:, :H], in_=a_w[og * 128:(og + 1) * 128, :])
        for hg in range(2):
            sl = (slice(None), slice(hg * 4, (hg + 1) * 4), slice(og * 128, (og + 1) * 128))
            transpose_split(wl[:, hg * 512:(hg + 1) * 512], 4,
                            WTh[sl[0], sl[1], sl[2]], WTl[sl[0], sl[1], sl[2]], ident)

    # W2T in DRAM scratch (f32r): w2t[k, o] = linear_out_w[o, k]
    w2t = dram.tile([2 * H, H], F32R)
    for og in range(HT):
        w2l = astage.tile([128, 2 * H], F32, tag="astage")
        nc.sync.dma_start(out=w2l, in_=a_w2[og * 128:(og + 1) * 128, :])
        for kg in range(4):
            stg = mixT_pool.tile([128, 4, 128], F32R, tag="w2stg")
            pst = transpose_to_psum(w2l[:, kg * 512:(kg + 1) * 512], 4, ident)
            nc.vector.tensor_copy(stg, pst)
            dst = w2t[kg * 512:(kg + 1) * 512, og * 128:(og + 1) * 128]
            nc.sync.dma_start(out=dst.rearrange("(a p) o -> p a o", p=128), in_=stg)

    # ---------------- per batch ----------------
    for b in range(BL):
        # cTh/cTl[p, kt, s]: bf16 hi/lo of context[b, s, kt*128+p]
        cTh = ct_pool.tile([128, HT, S], BF16, tag="cth")
        cTl = ct_pool.tile([128, HT, S], BF16, tag="ctl")
        for st in range(ST):
            cl = loadc.tile([128, H], F32, tag="cload")
            nc.sync.dma_start(out=cl, in_=a_context[b, st * 128:(st + 1) * 128, :])
            for hg in range(2):
                sl1, sl2 = slice(hg * 4, (hg + 1) * 4), slice(st * 128, (st + 1) * 128)
                transpose_split(cl[:, hg * 512:(hg + 1) * 512], 4,
                                cTh[:, sl1, sl2], cTl[:, sl1, sl2], ident)

        for ck in range(N_CHUNKS):
            t0 = ck * T_CHK

            # --- A: oTh/oTl[p, kt, t]: bf16 hi/lo of output[b, t0+t, kt*128+p] ---
            oTh = outT_pool.tile([128, HT, T_CHK], BF16, tag="oth")
            oTl = outT_pool.tile([128, HT, T_CHK], BF16, tag="otl")
            for tsub in range(TSUBS):
                ld = loadout.tile([128, H], F32, tag="oload")
                nc.sync.dma_start(
                    out=ld, in_=a_output[b, t0 + tsub * 128:t0 + (tsub + 1) * 128, :])
                for hg in range(2):
                    sl1 = slice(hg * 4, (hg + 1) * 4)
                    sl2 = slice(tsub * 128, (tsub + 1) * 128)
                    transpose_split(ld[:, hg * 512:(hg + 1) * 512], 4,
                                    oTh[:, sl1, sl2], oTl[:, sl1, sl2], ident)

            # --- B: q via 3-pass bf16 hi/lo; evacuate to qT f32 + q_hi/q_lo bf16 ---
            qT = qT_pool.tile([128, HT, T_CHK], F32, tag="qt")
            q_hi = qT_pool.tile([128, HT, T_CHK], BF16, tag="qhi")
            q_lo = qT_pool.tile([128, HT, T_CHK], BF16, tag="qlo")
            for og in range(4):
                psq = ps_mm.tile([128, 1024], F32, tag="mm")
                for oi in range(2):
                    ot = og * 2 + oi
                    tgt = psq[:, oi * 512:oi * 512 + T_CHK]
                    for kt in range(HT):
                        passes = [(WTh, oTh), (WTh, oTl), (WTl, oTh)]
                        for pi, (wsrc, osrc) in enumerate(passes):
                            nc.tensor.matmul(
                                tgt,
                                wsrc[:, kt, ot * 128:(ot + 1) * 128],
                                osrc[:, kt, :],
                                start=(kt == 0 and pi == 0),
                                stop=(kt == HT - 1 and pi == 2))
                view = psq.rearrange("p (a c) -> p a c", a=2)[:, :, :T_CHK]
                osl = slice(og * 2, og * 2 + 2)
                nc.vector.tensor_copy(qT[:, osl, :], view)
                nc.scalar.copy(q_hi[:, osl, :], view)
                nc.vector.tensor_tensor(q_lo[:, osl, :], view, q_hi[:, osl, :],
                                        op=ALU.subtract)

            # --- C: scores (3-pass bf16 hi/lo) + softmax + attn out + attnT ---
            attnT = attnT_pool.tile([128, ST, T_CHK], F32R)
            for tsub in range(TSUBS):
                r0 = t0 + tsub * 128
                tsl = slice(tsub * 128, (tsub + 1) * 128)
                ps_a = ps_mm.tile([128, 1024], F32, tag="mm")
                ps_b = ps_mm.tile([128, 1024], F32, tag="mm")
                for kt in range(HT):
                    passes = [(q_hi, cTh), (q_hi, cTl), (q_lo, cTh)]
                    for pi, (qsrc, csrc) in enumerate(passes):
                        qtile = qsrc[:, kt, tsl]
                        for sc in range(4):
                            tgt = ps_a if sc < 2 else ps_b
                            nc.tensor.matmul(
                                tgt[:, (sc % 2) * 512:(sc % 2 + 1) * 512],
                                qtile,
                                csrc[:, kt, sc * 512:(sc + 1) * 512],
                                start=(kt == 0 and pi == 0),
                                stop=(kt == HT - 1 and pi == 2))
                # evacuate psum quickly, then softmax out of SBUF
                ast = astage.tile([128, S], F32, tag="astage")
                nc.vector.tensor_copy(ast[:, :1024], ps_a)
                nc.scalar.copy(ast[:, 1024:], ps_b)
                nm = small.tile([128, 1], F32, tag="sm")
                nc.vector.tensor_reduce(nm, ast, axis=AX.X, op=ALU.max, negate=True)
                lsum = small.tile([128, 1], F32, tag="sm")
                nc.scalar.activation(ast, ast, AF.Exp, bias=nm, scale=1.0,
                                     accum_out=lsum)
                rinv = small.tile([128, 1], F32, tag="sm")
                nc.vector.reciprocal(rinv, lsum)
                nc.vector.tensor_scalar_mul(ast, ast, rinv)
                nc.sync.dma_start(out=a_attn[b, r0:r0 + 128, :], in_=ast)
                for sg in range(4):
                    transpose_pack(ast[:, sg * 512:(sg + 1) * 512], 4,
                                   attnT[:, sg * 4:(sg + 1) * 4, tsl], ident)

            # --- D: mix natural [t,h] (f32r), then transpose to combT[k,t] ---
            psm = [ps_mm.tile([128, 1024], F32, tag="mm", name=f"psm{i}")
                   for i in range(TSUBS)]
            for st in range(ST):
                cm = loadc.tile([128, H], F32R, tag="cload")
                nc.gpsimd.dma_start(out=cm, in_=a_context[b, st * 128:(st + 1) * 128, :])
                for tsub in range(TSUBS):
                    lhsT = attnT[:, st, tsub * 128:(tsub + 1) * 128]
                    for nchk in range(2):
                        nc.tensor.matmul(
                            psm[tsub][:, nchk * 512:(nchk + 1) * 512],
                            lhsT,
                            cm[:, nchk * 512:(nchk + 1) * 512],
                            start=(st == 0), stop=(st == ST - 1))
            combT = mixT_pool.tile([128, KT2, T_CHK], F32R, tag="combT")
            for tsub in range(TSUBS):
                mn = mixnat_pool.tile([128, H], F32, tag="mixnat")
                nc.vector.tensor_copy(mn, psm[tsub])
                for hg in range(2):
                    transpose_pack(mn[:, hg * 512:(hg + 1) * 512], 4,
                                   combT[:, hg * 4:(hg + 1) * 4,
                                         tsub * 128:(tsub + 1) * 128], ident)

            # --- E: linear_out (f32r) + bias + tanh ---
            # q half of combT: rounded copy of qT (f32 -> f32r)
            nc.vector.tensor_copy(combT[:, HT:, :], qT)
            pso = [ps_mm.tile([128, 1024], F32, tag="mm", name=f"pso{i}")
                   for i in range(TSUBS)]
            for kt in range(KT2):
                w2s = w2s_pool.tile([128, H], F32R, tag="w2s")
                nc.sync.dma_start(out=w2s, in_=w2t[kt * 128:(kt + 1) * 128, :])
                for tsub in range(TSUBS):
                    lhsT = combT[:, kt, tsub * 128:(tsub + 1) * 128]
                    for nchk in range(2):
                        nc.tensor.matmul(
                            pso[tsub][:, nchk * 512:(nchk + 1) * 512],
                            lhsT,
                            w2s[:, nchk * 512:(nchk + 1) * 512],
                            start=(kt == 0), stop=(kt == KT2 - 1))
            for tsub in range(TSUBS):
                nc.vector.tensor_tensor(pso[tsub], pso[tsub], b2bc, op=ALU.add)
                ost = ostage.tile([128, H], F32, tag="ostage")
                nc.scalar.activation(ost, pso[tsub], AF.Tanh)
                r0 = t0 + tsub * 128
                nc.sync.dma_start(out=a_out[b, r0:r0 + 128, :], in_=ost)


def _split_sync_waits(nc):
    """This walrus/ISA build accepts at most ONE sync-wait command per
    instruction, but Tile's sem-assigner can attach several (phase-first
    instructions, kernel-tail drain).  Split: keep the last wait on the
    instruction, hoist the rest onto same-engine NoOps inserted just before."""
    n_split = 0
    for fn in nc.m.functions:
        for blk in fn.blocks:
            insts = blk.instructions
            out = []
            changed = False
            for inst in insts:
                si = inst.sync_info
                waits = list(si.on_wait) if (si and si.on_wait) else []
                if len(waits) > 1:
                    for w in waits[:-1]:
                        nop = mybir.InstNoOp(
                            name=f"waitsplit-{nc.next_id()}",
                            engine=inst.engine,
                            sync_info=mybir.SyncInfo(on_wait=[w], on_update=[]),
                        )
                        out.append(nop)
                    inst.sync_info = mybir.SyncInfo(
                        on_wait=[waits[-1]], on_update=list(si.on_update or []))
                    n_split += 1
                    changed = True
                out.append(inst)
            if changed:
                blk.instructions = out
    return n_split


_CACHED_NC = {}


def _build(split_waits=True):
    if split_waits in _CACHED_NC:
        return _CACHED_NC[split_waits]
    nc = bass.Bass("TRN2", target_bir_lowering=False, debug=False)
    io = {
        "output": nc.dram_tensor("output", [BL, T, H], F32, kind="ExternalInput").ap(),
        "context": nc.dram_tensor("context", [BL, S, H], F32, kind="ExternalInput").ap(),
        "attn_weight": nc.dram_tensor("attn_weight", [H, H], F32, kind="ExternalInput").ap(),
        "linear_out_w": nc.dram_tensor("linear_out_w", [H, 2 * H], F32, kind="ExternalInput").ap(),
        "linear_out_b": nc.dram_tensor("linear_out_b", [H], F32, kind="ExternalInput").ap(),
        "out": nc.dram_tensor("out", [BL, T, H], F32, kind="ExternalOutput").ap(),
        "attn": nc.dram_tensor("attn", [BL, T, S], F32, kind="ExternalOutput").ap(),
    }
    with tile.TileContext(nc) as tc:
        with ExitStack() as ctx:
            _emit(ctx, tc, io)
    if split_waits:
        # CoreSim can't execute the bare NoOps; only split for the HW path.
        _split_sync_waits(nc)
    _CACHED_NC[split_waits] = nc
    return nc


def make_in_maps(inputs):
    in_maps = []
    for c in range(NCORES):
        b0 = c * BL
        in_maps.append({
            "output": np.ascontiguousarray(inputs["output"][b0:b0 + BL], dtype=np.float32),
            "context": np.ascontiguousarray(inputs["context"][b0:b0 + BL], dtype=np.float32),
            "attn_weight": np.ascontiguousarray(inputs["attn_weight"], dtype=np.float32),
            "linear_out_w": np.ascontiguousarray(inputs["linear_out_w"], dtype=np.float32),
            "linear_out_b": np.ascontiguousarray(inputs["linear_out_b"], dtype=np.float32),
        })
    return in_maps


LAST_RESULT = None


def kernel(**inputs):
    global LAST_RESULT
    nc = _build()
    in_maps = make_in_maps(inputs)
    trace = os.environ.get("KERNEL_TRACE", "0") == "1"
    res = bass_utils.run_bass_kernel_spmd(
        nc, in_maps, core_ids=list(range(NCORES)), trace=trace)
    LAST_RESULT = res
    out = np.concatenate([r["out"] for r in res.results], axis=0)
    attn = np.concatenate([r["attn"] for r in res.results], axis=0)
    return out, attn


if __name__ == "__main__":
    rng = np.random.default_rng(0)
    inputs = {
        "output": rng.standard_normal((B, T, H), dtype=np.float32),
        "context": rng.standard_normal((B, S, H), dtype=np.float32),
        "attn_weight": (rng.standard_normal((H, H)) / np.sqrt(H)).astype(np.float32),
        "linear_out_w": (rng.standard_normal((H, 2 * H)) / np.sqrt(2 * H)).astype(np.float32),
        "linear_out_b": (rng.standard_normal(H) * 0.01).astype(np.float32),
    }
    out, attn = kernel(**inputs)
    print("out", out.shape, "attn", attn.shape)


# revision 16
# speedup vs baseline: 1.0998x; 1.0927x over previous
"""Trainium2 Bass kernel for a cross-attention nn.Module.

Computes, for inputs (all fp32):
    q      = output @ attn_weight.T              [B,T,H]
    attn   = softmax(q @ context.T, axis=-1)     [B,T,S]
    mix    = attn @ context                      [B,T,H]
    out    = tanh(concat(mix, q) @ linear_out_w.T + linear_out_b)
Returns (out, attn).

Sharding: data-parallel over batch, 2 batches per NeuronCore x 8 cores,
no collectives.  Inside each core:
  - q and scores matmuls run in true fp32 (softmax is extremely sensitive:
    near-one-hot rows; bf16/tf32 scores give absmax errors ~0.02-0.16).
  - mix and linear_out matmuls run as float32r (full PE rate at N>=256).
  - layouts: contraction dims must sit on SBUF partitions, so context is
    PE-transposed once per batch (cT), attn is transposed per tile (attnT),
    mix is computed in natural [t,h] layout then transposed for linear_out.
"""

import os
import sys
import types
import numpy as np
from contextlib import ExitStack

import concourse.bass as bass
import concourse.mybir as mybir
import concourse.tile as tile
from concourse import bass_utils
from concourse.masks import make_identity


def _ensure_ntff_hook():
    """This deployment's antenv package lacks axon_hooks, which
    run_bass_kernel_spmd(trace=True) imports under axon.  Register a shim that
    drives NTFF profiling via ctypes into libaxon_pjrt.so (same contract as
    trn_agent_boot._ntff_profile_via_ctypes)."""
    try:
        from antenv.axon_hooks import get_axon_ntff_profile_hook  # noqa: F401
        return
    except ImportError:
        pass
    import contextlib
    import ctypes

    so_path = "/opt/axon/libaxon_pjrt.so"
    hook = None
    if os.path.exists(so_path):
        lib = ctypes.CDLL(so_path)
        if hasattr(lib, "axon_start_nrt_profile"):
            lib.axon_start_nrt_profile.argtypes = [
                ctypes.POINTER(ctypes.c_int64), ctypes.c_size_t]
            lib.axon_start_nrt_profile.restype = ctypes.c_int64
            lib.axon_stop_nrt_profile.argtypes = [ctypes.c_char_p]
            lib.axon_stop_nrt_profile.restype = ctypes.c_int64

            @contextlib.contextmanager
            def _hook(output_dir, device_ids):
                import jax
                jax.devices()
                if device_ids:
                    ids = (ctypes.c_int64 * len(device_ids))(*device_ids)
                    rc = lib.axon_start_nrt_profile(ids, len(device_ids))
                else:
                    rc = lib.axon_start_nrt_profile(None, 0)
                if rc != 0:
                    raise RuntimeError(f"axon_start_nrt_profile rc={rc}")
                try:
                    yield
                finally:
                    n = lib.axon_stop_nrt_profile(str(output_dir).encode())
                    print(f"ntff profile: {n} file(s) -> {output_dir}")

            hook = _hook

    mod = types.ModuleType("antenv.axon_hooks")
    mod._hook = hook
    mod.get_axon_ntff_profile_hook = lambda: mod._hook
    mod.set_axon_ntff_profile_hook = lambda h: setattr(mod, "_hook", h)
    sys.modules["antenv.axon_hooks"] = mod
    import antenv
    antenv.axon_hooks = mod


_ensure_ntff_hook()

F32 = mybir.dt.float32
F32R = mybir.dt.float32r
BF16 = mybir.dt.bfloat16
AF = mybir.ActivationFunctionType
ALU = mybir.AluOpType
AX = mybir.AxisListType

B, T, S, H = 16, 1024, 2048, 1024
NCORES = 8
BL = B // NCORES            # batches per core
T_CHK = 256                 # t rows per pipeline chunk
N_CHUNKS = T // T_CHK       # 4 per batch
TSUBS = T_CHK // 128        # 2 t-tiles per chunk
HT = H // 128               # 8 feature tiles
ST = S // 128               # 16 context-position tiles
KT2 = 2 * H // 128          # 16 contraction tiles for linear_out


def _emit(ctx: ExitStack, tc: "tile.TileContext", io: dict):
    nc = tc.nc
    a_out, a_attn = io["out"], io["attn"]
    a_output, a_context = io["output"], io["context"]
    a_w, a_w2, a_b2 = io["attn_weight"], io["linear_out_w"], io["linear_out_b"]

    const = ctx.enter_context(tc.tile_pool(name="const", bufs=1))
    ct_pool = ctx.enter_context(tc.tile_pool(name="ct", bufs=1))
    loadc = ctx.enter_context(tc.tile_pool(name="loadc", bufs=2))
    loadout = ctx.enter_context(tc.tile_pool(name="loadout", bufs=1))
    outT_pool = ctx.enter_context(tc.tile_pool(name="outT", bufs=1))
    q_pool = ctx.enter_context(tc.tile_pool(name="q", bufs=2))
    attnT_pool = ctx.enter_context(tc.tile_pool(name="attnT", bufs=1))
    mixnat_pool = ctx.enter_context(tc.tile_pool(name="mixnat", bufs=1))
    mixT_pool = ctx.enter_context(tc.tile_pool(name="mixT", bufs=1))
    astage = ctx.enter_context(tc.tile_pool(name="astage", bufs=2))
    ostage = ctx.enter_context(tc.tile_pool(name="ostage", bufs=1))
    w2s_pool = ctx.enter_context(tc.tile_pool(name="w2s", bufs=2))
    small = ctx.enter_context(tc.tile_pool(name="small", bufs=3))
    dram = ctx.enter_context(tc.tile_pool(name="dram", bufs=1, space="DRAM"))
    ps_mm = ctx.enter_context(tc.tile_pool(name="ps_mm", bufs=3, space="PSUM"))
    ps_tr = ctx.enter_context(tc.tile_pool(name="ps_tr", bufs=2, space="PSUM"))

    def transpose_to_psum(src_ap, n, ident):
        assert n <= 4
        pst = ps_tr.tile([128, 4, 128], F32, tag="tr")
        for i in range(n):
            nc.tensor.transpose(pst[:, i, :], src_ap[:, i * 128:(i + 1) * 128], ident)
        return pst

    def transpose_pack(src_ap, n, dst_ap, ident):
        pst = transpose_to_psum(src_ap, n, ident)
        nc.vector.tensor_copy(dst_ap, pst[:, :n, :])

    def transpose_split(src_ap, n, dst_hi, dst_lo, ident):
        pst = transpose_to_psum(src_ap, n, ident)
        nc.scalar.copy(dst_hi, pst[:, :n, :])
        nc.vector.tensor_tensor(dst_lo, pst[:, :n, :], dst_hi, op=ALU.subtract)

    # ---------------- phase 0: constants ----------------
    ident = const.tile([128, 128], F32)
    make_identity(nc, ident)

    b2bc = const.tile([128, H], F32)
    nc.gpsimd.dma_start(out=b2bc, in_=a_b2.partition_broadcast(128))

    WTh = const.tile([128, HT, H], BF16)
    WTl = const.tile([128, HT, H], BF16)
    for og in range(HT):
        wl = astage.tile([128, 2 * H], F32, tag="astage")
        nc.sync.dma_start(out=wl[:, :H], in_=a_w[og * 128:(og + 1) * 128, :])
        for hg in range(2):
            s1 = slice(hg * 4, (hg + 1) * 4)
            s2 = slice(og * 128, (og + 1) * 128)
            transpose_split(wl[:, hg * 512:(hg + 1) * 512], 4,
                            WTh[:, s1, s2], WTl[:, s1, s2], ident)

    w2t = dram.tile([2 * H, H], F32R)
    for og in range(HT):
        w2l = astage.tile([128, 2 * H], F32, tag="astage")
        nc.sync.dma_start(out=w2l, in_=a_w2[og * 128:(og + 1) * 128, :])
        for kg in range(4):
            stg = mixnat_pool.tile([128, 4, 128], F32R, tag="mixnat")
            pst = transpose_to_psum(w2l[:, kg * 512:(kg + 1) * 512], 4, ident)
            nc.vector.tensor_copy(stg, pst)
            dst = w2t[kg * 512:(kg + 1) * 512, og * 128:(og + 1) * 128]
            nc.sync.dma_start(out=dst.rearrange("(a p) o -> p a o", p=128), in_=stg)

    # ---------------- pipelined chunk jobs ----------------
    jobs = [(b, ck) for b in range(BL) for ck in range(N_CHUNKS)]
    cT = {}     # b -> (cTh, cTl)
    qbuf = {}   # job idx -> (q_hi, q_lo, q_r)
    astash = {} # job idx -> [ast per tsub]

    def build_ct(b):
        cTh = ct_pool.tile([128, HT, S], BF16, tag="cth", name=f"cth{b}")
        cTl = ct_pool.tile([128, HT, S], BF16, tag="ctl", name=f"ctl{b}")
        for st in range(ST):
            cl = loadc.tile([128, H], F32, tag="cload")
            nc.sync.dma_start(out=cl, in_=a_context[b, st * 128:(st + 1) * 128, :])
            for hg in range(2):
                s1, s2 = slice(hg * 4, (hg + 1) * 4), slice(st * 128, (st + 1) * 128)
                transpose_split(cl[:, hg * 512:(hg + 1) * 512], 4,
                                cTh[:, s1, s2], cTl[:, s1, s2], ident)
        cT[b] = (cTh, cTl)

    def phase_ab(idx):
        """Load + transpose output chunk, compute q, evacuate to bf16 hi/lo + f32r."""
        b, ck = jobs[idx]
        t0 = ck * T_CHK
        oTh = outT_pool.tile([128, HT, T_CHK], BF16, tag="oth", name=f"oth{idx}")
        oTl = outT_pool.tile([128, HT, T_CHK], BF16, tag="otl", name=f"otl{idx}")
        for tsub in range(TSUBS):
            ld = loadout.tile([128, H], F32, tag="oload")
            nc.sync.dma_start(
                out=ld, in_=a_output[b, t0 + tsub * 128:t0 + (tsub + 1) * 128, :])
            for hg in range(2):
                s1 = slice(hg * 4, (hg + 1) * 4)
                s2 = slice(tsub * 128, (tsub + 1) * 128)
                transpose_split(ld[:, hg * 512:(hg + 1) * 512], 4,
                                oTh[:, s1, s2], oTl[:, s1, s2], ident)
        q_hi = q_pool.tile([128, HT, T_CHK], BF16, tag="qhi", name=f"qhi{idx}", bufs=1)
        q_lo = q_pool.tile([128, HT, T_CHK], BF16, tag="qlo", name=f"qlo{idx}", bufs=1)
        q_r = q_pool.tile([128, HT, T_CHK], F32R, tag="qr", name=f"qr{idx}")
        for og in range(4):
            psq = ps_mm.tile([128, 1024], F32, tag="mm")
            for oi in range(2):
                ot = og * 2 + oi
                tgt = psq[:, oi * 512:oi * 512 + T_CHK]
                for kt in range(HT):
                    passes = [(WTh, oTh), (WTh, oTl), (WTl, oTh)]
                    for pi, (wsrc, osrc) in enumerate(passes):
                        nc.tensor.matmul(
                            tgt,
                            wsrc[:, kt, ot * 128:(ot + 1) * 128],
                            osrc[:, kt, :],
                            start=(kt == 0 and pi == 0),
                            stop=(kt == HT - 1 and pi == 2))
            view = psq.rearrange("p (a c) -> p a c", a=2)[:, :, :T_CHK]
            osl = slice(og * 2, og * 2 + 2)
            nc.scalar.copy(q_hi[:, osl, :], view)
            nc.vector.tensor_tensor(q_lo[:, osl, :], view, q_hi[:, osl, :],
                                    op=ALU.subtract)
            nc.vector.tensor_copy(q_r[:, osl, :], view)
        qbuf[idx] = (q_hi, q_lo, q_r)

    def phase_scores(idx):
        b, ck = jobs[idx]
        t0 = ck * T_CHK
        q_hi, q_lo, _ = qbuf[idx]
        cTh, cTl = cT[b]
        asts = []
        for tsub in range(TSUBS):
            r0 = t0 + tsub * 128
            tsl = slice(tsub * 128, (tsub + 1) * 128)
            ps_a = ps_mm.tile([128, 1024], F32, tag="mm")
            ps_b = ps_mm.tile([128, 1024], F32, tag="mm")
            for kt in range(HT):
                passes = [(q_hi, cTh), (q_hi, cTl), (q_lo, cTh)]
                for pi, (qsrc, csrc) in enumerate(passes):
                    qtile = qsrc[:, kt, tsl]
                    for sc in range(4):
                        tgt = ps_a if sc < 2 else ps_b
                        nc.tensor.matmul(
                            tgt[:, (sc % 2) * 512:(sc % 2 + 1) * 512],
                            qtile,
                            csrc[:, kt, sc * 512:(sc + 1) * 512],
                            start=(kt == 0 and pi == 0),
                            stop=(kt == HT - 1 and pi == 2))
            ast = astage.tile([128, S], F32, tag="astage")
            nc.vector.tensor_copy(ast[:, :1024], ps_a)
            nc.scalar.copy(ast[:, 1024:], ps_b)
            stats = small.tile([128, 4], F32, tag="sm")
            nm, lsum, rinv = stats[:, 0:1], stats[:, 1:2], stats[:, 2:3]
            nc.vector.tensor_reduce(nm, ast, axis=AX.X, op=ALU.max, negate=True)
            nc.scalar.activation(ast, ast, AF.Exp, bias=nm, scale=1.0, accum_out=lsum)
            nc.vector.reciprocal(rinv, lsum)
            nc.vector.tensor_scalar_mul(ast, ast, rinv)
            nc.sync.dma_start(out=a_attn[b, r0:r0 + 128, :], in_=ast)
            asts.append(ast)
        astash[idx] = asts

    def phase_tail(idx):
        b, ck = jobs[idx]
        t0 = ck * T_CHK
        _, _, q_r = qbuf[idx]
        asts = astash.pop(idx)
        # attnT transposes (softmax latency hidden by interleaved next-chunk work)
        attnT = attnT_pool.tile([128, ST, T_CHK], F32R, tag="attnT", name=f"attnT{idx}")
        for tsub in range(TSUBS):
            tsl = slice(tsub * 128, (tsub + 1) * 128)
            for sg in range(4):
                transpose_pack(asts[tsub][:, sg * 512:(sg + 1) * 512], 4,
                               attnT[:, sg * 4:(sg + 1) * 4, tsl], ident)
        # mix natural [t,h] (f32r)
        psm = [ps_mm.tile([128, 1024], F32, tag="mm", name=f"psm{idx}_{i}")
               for i in range(TSUBS)]
        for st in range(ST):
            cm = loadc.tile([128, H], F32R, tag="cload")
            nc.gpsimd.dma_start(out=cm, in_=a_context[b, st * 128:(st + 1) * 128, :])
            for tsub in range(TSUBS):
                lhsT = attnT[:, st, tsub * 128:(tsub + 1) * 128]
                for nchk in range(2):
                    nc.tensor.matmul(
                        psm[tsub][:, nchk * 512:(nchk + 1) * 512],
                        lhsT,
                        cm[:, nchk * 512:(nchk + 1) * 512],
                        start=(st == 0), stop=(st == ST - 1))
        mixT = mixT_pool.tile([128, HT, T_CHK], F32R, tag="mixT", name=f"mixT{idx}")
        mns = []
        for tsub in range(TSUBS):
            mn = mixnat_pool.tile([128, H], F32, tag="mixnat")
            nc.vector.tensor_copy(mn, psm[tsub])
            mns.append(mn)
        for tsub in range(TSUBS):
            for hg in range(2):
                transpose_pack(mns[tsub][:, hg * 512:(hg + 1) * 512], 4,
                               mixT[:, hg * 4:(hg + 1) * 4,
                                    tsub * 128:(tsub + 1) * 128], ident)
        # linear_out (f32r) + bias + tanh
        pso = [ps_mm.tile([128, 1024], F32, tag="mm", name=f"pso{idx}_{i}")
               for i in range(TSUBS)]
        for kt in range(KT2):
            w2s = w2s_pool.tile([128, H], F32R, tag="w2s")
            nc.sync.dma_start(out=w2s, in_=w2t[kt * 128:(kt + 1) * 128, :])
            for tsub in range(TSUBS):
                if kt < HT:
                    lhsT = mixT[:, kt, tsub * 128:(tsub + 1) * 128]
                else:
                    lhsT = q_r[:, kt - HT, tsub * 128:(tsub + 1) * 128]
                for nchk in range(2):
                    nc.tensor.matmul(
                        pso[tsub][:, nchk * 512:(nchk + 1) * 512],
                        lhsT,
                        w2s[:, nchk * 512:(nchk + 1) * 512],
                        start=(kt == 0), stop=(kt == KT2 - 1))
        for tsub in range(TSUBS):
            nc.vector.tensor_tensor(pso[tsub], pso[tsub], b2bc, op=ALU.add)
            ost = ostage.tile([128, H], F32, tag="ostage")
            nc.scalar.activation(ost, pso[tsub], AF.Tanh)
            r0 = t0 + tsub * 128
            nc.sync.dma_start(out=a_out[b, r0:r0 + 128, :], in_=ost)
        del qbuf[idx]

    build_ct(0)
    phase_ab(0)
    for idx, (b, ck) in enumerate(jobs):
        phase_scores(idx)
        if idx + 1 < len(jobs):
            nb, nck = jobs[idx + 1]
            phase_ab(idx + 1)
            if nb != b:
                build_ct(nb)
        phase_tail(idx)


def _split_sync_waits(nc):
    """This walrus/ISA build accepts at most ONE sync-wait command per
    instruction, but Tile's sem-assigner can attach several (phase-first
    instructions, kernel-tail drain).  Split: keep the last wait on the
    instruction, hoist the rest onto same-engine NoOps inserted just before."""
    n_split = 0
    for fn in nc.m.functions:
        for blk in fn.blocks:
            insts = blk.instructions
            out = []
            changed = False
            for inst in insts:
                si = inst.sync_info
                waits = list(si.on_wait) if (si and si.on_wait) else []
                if len(waits) > 1:
                    for w in waits[:-1]:
                        nop = mybir.InstNoOp(
                            name=f"waitsplit-{nc.next_id()}",
                            engine=inst.engine,
                            sync_info=mybir.SyncInfo(on_wait=[w], on_update=[]),
                        )
                        out.append(nop)
                    inst.sync_info = mybir.SyncInfo(
                        on_wait=[waits[-1]], on_update=list(si.on_update or []))
                    n_split += 1
                    changed = True
                out.append(inst)
            if changed:
                blk.instructions = out
    return n_split


_CACHED_NC = {}


def _build(split_waits=True):
    if split_waits in _CACHED_NC:
        return _CACHED_NC[split_waits]
    nc = bass.Bass("TRN2", target_bir_lowering=False, debug=False)
    io = {
        "output": nc.dram_tensor("output", [BL, T, H], F32, kind="ExternalInput").ap(),
        "context": nc.dram_tensor("context", [BL, S, H], F32, kind="ExternalInput").ap(),
        "attn_weight": nc.dram_tensor("attn_weight", [H, H], F32, kind="ExternalInput").ap(),
        "linear_out_w": nc.dram_tensor("linear_out_w", [H, 2 * H], F32, kind="ExternalInput").ap(),
        "linear_out_b": nc.dram_tensor("linear_out_b", [H], F32, kind="ExternalInput").ap(),
        "out": nc.dram_tensor("out", [BL, T, H], F32, kind="ExternalOutput").ap(),
        "attn": nc.dram_tensor("attn", [BL, T, S], F32, kind="ExternalOutput").ap(),
    }
    with tile.TileContext(nc) as tc:
        with ExitStack() as ctx:
            _emit(ctx, tc, io)
    if split_waits:
        # CoreSim can't execute the bare NoOps; only split for the HW path.
        _split_sync_waits(nc)
    _CACHED_NC[split_waits] = nc
    return nc


def make_in_maps(inputs):
    in_maps = []
    for c in range(NCORES):
        b0 = c * BL
        in_maps.append({
            "output": np.ascontiguousarray(inputs["output"][b0:b0 + BL], dtype=np.float32),
            "context": np.ascontiguousarray(inputs["context"][b0:b0 + BL], dtype=np.float32),
            "attn_weight": np.ascontiguousarray(inputs["attn_weight"], dtype=np.float32),
            "linear_out_w": np.ascontiguousarray(inputs["linear_out_w"], dtype=np.float32),
            "linear_out_b": np.ascontiguousarray(inputs["linear_out_b"], dtype=np.float32),
        })
    return in_maps


LAST_RESULT = None


def kernel(**inputs):
    global LAST_RESULT
    nc = _build()
    in_maps = make_in_maps(inputs)
    trace = os.environ.get("KERNEL_TRACE", "0") == "1"
    res = bass_utils.run_bass_kernel_spmd(
        nc, in_maps, core_ids=list(range(NCORES)), trace=trace)
    LAST_RESULT = res
    out = np.concatenate([r["out"] for r in res.results], axis=0)
    attn = np.concatenate([r["attn"] for r in res.results], axis=0)
    return out, attn


if __name__ == "__main__":
    rng = np.random.default_rng(0)
    inputs = {
        "output": rng.standard_normal((B, T, H), dtype=np.float32),
        "context": rng.standard_normal((B, S, H), dtype=np.float32),
        "attn_weight": (rng.standard_normal((H, H)) / np.sqrt(H)).astype(np.float32),
        "linear_out_w": (rng.standard_normal((H, 2 * H)) / np.sqrt(2 * H)).astype(np.float32),
        "linear_out_b": (rng.standard_normal(H) * 0.01).astype(np.float32),
    }
    out, attn = kernel(**inputs)
    print("out", out.shape, "attn", attn.shape)


# revision 17
# speedup vs baseline: 1.1082x; 1.0077x over previous
"""Trainium2 Bass kernel for a cross-attention nn.Module.

Computes, for inputs (all fp32):
    q      = output @ attn_weight.T              [B,T,H]
    attn   = softmax(q @ context.T, axis=-1)     [B,T,S]
    mix    = attn @ context                      [B,T,H]
    out    = tanh(concat(mix, q) @ linear_out_w.T + linear_out_b)
Returns (out, attn).

Sharding: data-parallel over batch, 2 batches per NeuronCore x 8 cores,
no collectives.  Inside each core:
  - q and scores matmuls run in true fp32 (softmax is extremely sensitive:
    near-one-hot rows; bf16/tf32 scores give absmax errors ~0.02-0.16).
  - mix and linear_out matmuls run as float32r (full PE rate at N>=256).
  - layouts: contraction dims must sit on SBUF partitions, so context is
    PE-transposed once per batch (cT), attn is transposed per tile (attnT),
    mix is computed in natural [t,h] layout then transposed for linear_out.
"""

import os
import sys
import types
import numpy as np
from contextlib import ExitStack

import concourse.bass as bass
import concourse.mybir as mybir
import concourse.tile as tile
from concourse import bass_utils
from concourse.masks import make_identity


def _ensure_ntff_hook():
    """This deployment's antenv package lacks axon_hooks, which
    run_bass_kernel_spmd(trace=True) imports under axon.  Register a shim that
    drives NTFF profiling via ctypes into libaxon_pjrt.so (same contract as
    trn_agent_boot._ntff_profile_via_ctypes)."""
    try:
        from antenv.axon_hooks import get_axon_ntff_profile_hook  # noqa: F401
        return
    except ImportError:
        pass
    import contextlib
    import ctypes

    so_path = "/opt/axon/libaxon_pjrt.so"
    hook = None
    if os.path.exists(so_path):
        lib = ctypes.CDLL(so_path)
        if hasattr(lib, "axon_start_nrt_profile"):
            lib.axon_start_nrt_profile.argtypes = [
                ctypes.POINTER(ctypes.c_int64), ctypes.c_size_t]
            lib.axon_start_nrt_profile.restype = ctypes.c_int64
            lib.axon_stop_nrt_profile.argtypes = [ctypes.c_char_p]
            lib.axon_stop_nrt_profile.restype = ctypes.c_int64

            @contextlib.contextmanager
            def _hook(output_dir, device_ids):
                import jax
                jax.devices()
                if device_ids:
                    ids = (ctypes.c_int64 * len(device_ids))(*device_ids)
                    rc = lib.axon_start_nrt_profile(ids, len(device_ids))
                else:
                    rc = lib.axon_start_nrt_profile(None, 0)
                if rc != 0:
                    raise RuntimeError(f"axon_start_nrt_profile rc={rc}")
                try:
                    yield
                finally:
                    n = lib.axon_stop_nrt_profile(str(output_dir).encode())
                    print(f"ntff profile: {n} file(s) -> {output_dir}")

            hook = _hook

    mod = types.ModuleType("antenv.axon_hooks")
    mod._hook = hook
    mod.get_axon_ntff_profile_hook = lambda: mod._hook
    mod.set_axon_ntff_profile_hook = lambda h: setattr(mod, "_hook", h)
    sys.modules["antenv.axon_hooks"] = mod
    import antenv
    antenv.axon_hooks = mod


_ensure_ntff_hook()

F32 = mybir.dt.float32
F32R = mybir.dt.float32r
BF16 = mybir.dt.bfloat16
AF = mybir.ActivationFunctionType
ALU = mybir.AluOpType
AX = mybir.AxisListType

B, T, S, H = 16, 1024, 2048, 1024
NCORES = 8
BL = B // NCORES            # batches per core
T_CHK = 256                 # t rows per pipeline chunk
N_CHUNKS = T // T_CHK       # 4 per batch
TSUBS = T_CHK // 128        # 2 t-tiles per chunk
HT = H // 128               # 8 feature tiles
ST = S // 128               # 16 context-position tiles
KT2 = 2 * H // 128          # 16 contraction tiles for linear_out


def _emit(ctx: ExitStack, tc: "tile.TileContext", io: dict):
    nc = tc.nc
    a_out, a_attn = io["out"], io["attn"]
    a_output, a_context = io["output"], io["context"]
    a_w, a_w2, a_b2 = io["attn_weight"], io["linear_out_w"], io["linear_out_b"]

    const = ctx.enter_context(tc.tile_pool(name="const", bufs=1))
    ct_pool = ctx.enter_context(tc.tile_pool(name="ct", bufs=1))
    loadc = ctx.enter_context(tc.tile_pool(name="loadc", bufs=2))
    loadout = ctx.enter_context(tc.tile_pool(name="loadout", bufs=2))
    outT_pool = ctx.enter_context(tc.tile_pool(name="outT", bufs=1))
    q_pool = ctx.enter_context(tc.tile_pool(name="q", bufs=2))
    attnT_pool = ctx.enter_context(tc.tile_pool(name="attnT", bufs=1))
    mixnat_pool = ctx.enter_context(tc.tile_pool(name="mixnat", bufs=1))
    mixT_pool = ctx.enter_context(tc.tile_pool(name="mixT", bufs=1))
    astage = ctx.enter_context(tc.tile_pool(name="astage", bufs=2))
    ostage = ctx.enter_context(tc.tile_pool(name="ostage", bufs=1))
    w2s_pool = ctx.enter_context(tc.tile_pool(name="w2s", bufs=2))
    small = ctx.enter_context(tc.tile_pool(name="small", bufs=3))
    dram = ctx.enter_context(tc.tile_pool(name="dram", bufs=1, space="DRAM"))
    ps_mm = ctx.enter_context(tc.tile_pool(name="ps_mm", bufs=6, space="PSUM"))
    ps_tr = ctx.enter_context(tc.tile_pool(name="ps_tr", bufs=2, space="PSUM"))

    def transpose_to_psum(src_ap, n, ident):
        assert n <= 4
        pst = ps_tr.tile([128, 4, 128], F32, tag="tr")
        for i in range(n):
            nc.tensor.transpose(pst[:, i, :], src_ap[:, i * 128:(i + 1) * 128], ident)
        return pst

    def transpose_pack(src_ap, n, dst_ap, ident):
        pst = transpose_to_psum(src_ap, n, ident)
        nc.vector.tensor_copy(dst_ap, pst[:, :n, :])

    def transpose_split(src_ap, n, dst_hi, dst_lo, ident):
        pst = transpose_to_psum(src_ap, n, ident)
        nc.scalar.copy(dst_hi, pst[:, :n, :])
        nc.vector.tensor_tensor(dst_lo, pst[:, :n, :], dst_hi, op=ALU.subtract)

    # ---------------- phase 0: constants ----------------
    ident = const.tile([128, 128], F32)
    make_identity(nc, ident)

    b2bc = const.tile([128, H], F32)
    nc.gpsimd.dma_start(out=b2bc, in_=a_b2.partition_broadcast(128))

    WTh = const.tile([128, HT, H], BF16)
    WTl = const.tile([128, HT, H], BF16)
    for og in range(HT):
        wl = astage.tile([128, 2 * H], F32, tag="astage")
        nc.sync.dma_start(out=wl[:, :H], in_=a_w[og * 128:(og + 1) * 128, :])
        for hg in range(2):
            s1 = slice(hg * 4, (hg + 1) * 4)
            s2 = slice(og * 128, (og + 1) * 128)
            transpose_split(wl[:, hg * 512:(hg + 1) * 512], 4,
                            WTh[:, s1, s2], WTl[:, s1, s2], ident)

    w2t = dram.tile([2 * H, H], F32R)
    for og in range(HT):
        w2l = astage.tile([128, 2 * H], F32, tag="astage")
        nc.sync.dma_start(out=w2l, in_=a_w2[og * 128:(og + 1) * 128, :])
        for kg in range(4):
            stg = mixnat_pool.tile([128, 4, 128], F32R, tag="mixnat")
            pst = transpose_to_psum(w2l[:, kg * 512:(kg + 1) * 512], 4, ident)
            nc.vector.tensor_copy(stg, pst)
            dst = w2t[kg * 512:(kg + 1) * 512, og * 128:(og + 1) * 128]
            nc.sync.dma_start(out=dst.rearrange("(a p) o -> p a o", p=128), in_=stg)

    # ---------------- pipelined chunk jobs ----------------
    jobs = [(b, ck) for b in range(BL) for ck in range(N_CHUNKS)]
    cT = {}     # b -> (cTh, cTl)
    qbuf = {}   # job idx -> (q_hi, q_lo, q_r)
    astash = {} # job idx -> [ast per tsub]

    def build_ct(b):
        cTh = ct_pool.tile([128, HT, S], BF16, tag="cth", name=f"cth{b}")
        cTl = ct_pool.tile([128, HT, S], BF16, tag="ctl", name=f"ctl{b}")
        for st in range(ST):
            cl = loadc.tile([128, H], F32, tag="cload")
            nc.sync.dma_start(out=cl, in_=a_context[b, st * 128:(st + 1) * 128, :])
            for hg in range(2):
                s1, s2 = slice(hg * 4, (hg + 1) * 4), slice(st * 128, (st + 1) * 128)
                transpose_split(cl[:, hg * 512:(hg + 1) * 512], 4,
                                cTh[:, s1, s2], cTl[:, s1, s2], ident)
        cT[b] = (cTh, cTl)

    def phase_ab(idx):
        """Load + transpose output chunk, compute q, evacuate to bf16 hi/lo + f32r."""
        b, ck = jobs[idx]
        t0 = ck * T_CHK
        oTh = outT_pool.tile([128, HT, T_CHK], BF16, tag="oth", name=f"oth{idx}")
        oTl = outT_pool.tile([128, HT, T_CHK], BF16, tag="otl", name=f"otl{idx}")
        for tsub in range(TSUBS):
            ld = loadout.tile([128, H], F32, tag="oload")
            nc.sync.dma_start(
                out=ld, in_=a_output[b, t0 + tsub * 128:t0 + (tsub + 1) * 128, :])
            for hg in range(2):
                s1 = slice(hg * 4, (hg + 1) * 4)
                s2 = slice(tsub * 128, (tsub + 1) * 128)
                transpose_split(ld[:, hg * 512:(hg + 1) * 512], 4,
                                oTh[:, s1, s2], oTl[:, s1, s2], ident)
        q_hi = q_pool.tile([128, HT, T_CHK], BF16, tag="qhi", name=f"qhi{idx}", bufs=1)
        q_lo = q_pool.tile([128, HT, T_CHK], BF16, tag="qlo", name=f"qlo{idx}", bufs=1)
        q_r = q_pool.tile([128, HT, T_CHK], F32R, tag="qr", name=f"qr{idx}")
        for ot in range(HT):
            psq = ps_mm.tile([128, 512], F32, tag="mm")
            tgt = psq[:, :T_CHK]
            for kt in range(HT):
                passes = [(WTh, oTh), (WTh, oTl), (WTl, oTh)]
                for pi, (wsrc, osrc) in enumerate(passes):
                    nc.tensor.matmul(
                        tgt,
                        wsrc[:, kt, ot * 128:(ot + 1) * 128],
                        osrc[:, kt, :],
                        start=(kt == 0 and pi == 0),
                        stop=(kt == HT - 1 and pi == 2))
            osl = slice(ot, ot + 1)
            view = tgt.rearrange("p (a c) -> p a c", a=1)
            nc.scalar.copy(q_hi[:, osl, :], view)
            nc.vector.tensor_tensor(q_lo[:, osl, :], view, q_hi[:, osl, :],
                                    op=ALU.subtract)
            nc.vector.tensor_copy(q_r[:, osl, :], view)
        qbuf[idx] = (q_hi, q_lo, q_r)

    def phase_scores(idx):
        b, ck = jobs[idx]
        t0 = ck * T_CHK
        q_hi, q_lo, _ = qbuf[idx]
        cTh, cTl = cT[b]
        asts = []
        for tsub in range(TSUBS):
            r0 = t0 + tsub * 128
            tsl = slice(tsub * 128, (tsub + 1) * 128)
            pss = [ps_mm.tile([128, 512], F32, tag="mm", name=f"pss{tsub}_{i}")
                   for i in range(4)]
            for kt in range(HT):
                passes = [(q_hi, cTh), (q_hi, cTl), (q_lo, cTh)]
                for pi, (qsrc, csrc) in enumerate(passes):
                    qtile = qsrc[:, kt, tsl]
                    for sc in range(4):
                        nc.tensor.matmul(
                            pss[sc],
                            qtile,
                            csrc[:, kt, sc * 512:(sc + 1) * 512],
                            start=(kt == 0 and pi == 0),
                            stop=(kt == HT - 1 and pi == 2))
            ast = astage.tile([128, S], F32, tag="astage")
            for sc in range(4):
                eng = nc.vector if sc % 2 == 0 else nc.scalar
                if sc % 2 == 0:
                    nc.vector.tensor_copy(ast[:, sc * 512:(sc + 1) * 512], pss[sc])
                else:
                    nc.scalar.copy(ast[:, sc * 512:(sc + 1) * 512], pss[sc])
            stats = small.tile([128, 4], F32, tag="sm")
            nm, lsum, rinv = stats[:, 0:1], stats[:, 1:2], stats[:, 2:3]
            nc.vector.tensor_reduce(nm, ast, axis=AX.X, op=ALU.max, negate=True)
            nc.scalar.activation(ast, ast, AF.Exp, bias=nm, scale=1.0, accum_out=lsum)
            nc.vector.reciprocal(rinv, lsum)
            nc.vector.tensor_scalar_mul(ast, ast, rinv)
            nc.sync.dma_start(out=a_attn[b, r0:r0 + 128, :], in_=ast)
            asts.append(ast)
        astash[idx] = asts

    def phase_tail(idx):
        b, ck = jobs[idx]
        t0 = ck * T_CHK
        _, _, q_r = qbuf[idx]
        asts = astash.pop(idx)
        # attnT transposes (softmax latency hidden by interleaved next-chunk work)
        attnT = attnT_pool.tile([128, ST, T_CHK], F32R, tag="attnT", name=f"attnT{idx}")
        for tsub in range(TSUBS):
            tsl = slice(tsub * 128, (tsub + 1) * 128)
            for sg in range(4):
                transpose_pack(asts[tsub][:, sg * 512:(sg + 1) * 512], 4,
                               attnT[:, sg * 4:(sg + 1) * 4, tsl], ident)
        # mix natural [t,h] (f32r)
        psm = [ps_mm.tile([128, 512], F32, tag="mm", name=f"psm{idx}_{i}")
               for i in range(TSUBS * 2)]
        for st in range(ST):
            cm = loadc.tile([128, H], F32R, tag="cload")
            nc.gpsimd.dma_start(out=cm, in_=a_context[b, st * 128:(st + 1) * 128, :])
            for tsub in range(TSUBS):
                lhsT = attnT[:, st, tsub * 128:(tsub + 1) * 128]
                for nchk in range(2):
                    nc.tensor.matmul(
                        psm[tsub * 2 + nchk],
                        lhsT,
                        cm[:, nchk * 512:(nchk + 1) * 512],
                        start=(st == 0), stop=(st == ST - 1))
        mixT = mixT_pool.tile([128, HT, T_CHK], F32R, tag="mixT", name=f"mixT{idx}")
        mns = []
        for tsub in range(TSUBS):
            mn = mixnat_pool.tile([128, H], F32, tag="mixnat")
            nc.vector.tensor_copy(mn[:, :512], psm[tsub * 2])
            nc.scalar.copy(mn[:, 512:], psm[tsub * 2 + 1])
            mns.append(mn)
        # linear_out (f32r): q-half contraction first so PE stays busy while
        # mixT is being transposed; then the mixT half.  + bias + tanh
        pso = [ps_mm.tile([128, 512], F32, tag="mm", name=f"pso{idx}_{i}")
               for i in range(TSUBS * 2)]
        kt_order = list(range(HT, KT2)) + list(range(HT))
        emitted_mixT = False
        for kn, kt in enumerate(kt_order):
            if kt < HT and not emitted_mixT:
                for tsub in range(TSUBS):
                    for hg in range(2):
                        transpose_pack(mns[tsub][:, hg * 512:(hg + 1) * 512], 4,
                                       mixT[:, hg * 4:(hg + 1) * 4,
                                            tsub * 128:(tsub + 1) * 128], ident)
                emitted_mixT = True
            w2s = w2s_pool.tile([128, H], F32R, tag="w2s")
            nc.sync.dma_start(out=w2s, in_=w2t[kt * 128:(kt + 1) * 128, :])
            for tsub in range(TSUBS):
                if kt < HT:
                    lhsT = mixT[:, kt, tsub * 128:(tsub + 1) * 128]
                else:
                    lhsT = q_r[:, kt - HT, tsub * 128:(tsub + 1) * 128]
                for nchk in range(2):
                    nc.tensor.matmul(
                        pso[tsub * 2 + nchk],
                        lhsT,
                        w2s[:, nchk * 512:(nchk + 1) * 512],
                        start=(kn == 0), stop=(kn == KT2 - 1))
        for tsub in range(TSUBS):
            ost = ostage.tile([128, H], F32, tag="ostage")
            for nchk in range(2):
                nc.vector.tensor_tensor(pso[tsub * 2 + nchk], pso[tsub * 2 + nchk],
                                        b2bc[:, nchk * 512:(nchk + 1) * 512],
                                        op=ALU.add)
                nc.scalar.activation(ost[:, nchk * 512:(nchk + 1) * 512],
                                     pso[tsub * 2 + nchk], AF.Tanh)
            r0 = t0 + tsub * 128
            nc.sync.dma_start(out=a_out[b, r0:r0 + 128, :], in_=ost)
        del qbuf[idx]

    build_ct(0)
    phase_ab(0)
    for idx, (b, ck) in enumerate(jobs):
        phase_scores(idx)
        if idx + 1 < len(jobs):
            nb, nck = jobs[idx + 1]
            phase_ab(idx + 1)
            if nb != b:
                build_ct(nb)
        phase_tail(idx)


def _split_sync_waits(nc):
    """This walrus/ISA build accepts at most ONE sync-wait command per
    instruction, but Tile's sem-assigner can attach several (phase-first
    instructions, kernel-tail drain).  Split: keep the last wait on the
    instruction, hoist the rest onto same-engine NoOps inserted just before."""
    n_split = 0
    for fn in nc.m.functions:
        for blk in fn.blocks:
            insts = blk.instructions
            out = []
            changed = False
            for inst in insts:
                si = inst.sync_info
                waits = list(si.on_wait) if (si and si.on_wait) else []
                if len(waits) > 1:
                    for w in waits[:-1]:
                        nop = mybir.InstNoOp(
                            name=f"waitsplit-{nc.next_id()}",
                            engine=inst.engine,
                            sync_info=mybir.SyncInfo(on_wait=[w], on_update=[]),
                        )
                        out.append(nop)
                    inst.sync_info = mybir.SyncInfo(
                        on_wait=[waits[-1]], on_update=list(si.on_update or []))
                    n_split += 1
                    changed = True
                out.append(inst)
            if changed:
                blk.instructions = out
    return n_split


_CACHED_NC = {}


def _build(split_waits=True):
    if split_waits in _CACHED_NC:
        return _CACHED_NC[split_waits]
    nc = bass.Bass("TRN2", target_bir_lowering=False, debug=False)
    io = {
        "output": nc.dram_tensor("output", [BL, T, H], F32, kind="ExternalInput").ap(),
        "context": nc.dram_tensor("context", [BL, S, H], F32, kind="ExternalInput").ap(),
        "attn_weight": nc.dram_tensor("attn_weight", [H, H], F32, kind="ExternalInput").ap(),
        "linear_out_w": nc.dram_tensor("linear_out_w", [H, 2 * H], F32, kind="ExternalInput").ap(),
        "linear_out_b": nc.dram_tensor("linear_out_b", [H], F32, kind="ExternalInput").ap(),
        "out": nc.dram_tensor("out", [BL, T, H], F32, kind="ExternalOutput").ap(),
        "attn": nc.dram_tensor("attn", [BL, T, S], F32, kind="ExternalOutput").ap(),
    }
    with tile.TileContext(nc) as tc:
        with ExitStack() as ctx:
            _emit(ctx, tc, io)
    if split_waits:
        # CoreSim can't execute the bare NoOps; only split for the HW path.
        _split_sync_waits(nc)
    _CACHED_NC[split_waits] = nc
    return nc


def make_in_maps(inputs):
    in_maps = []
    for c in range(NCORES):
        b0 = c * BL
        in_maps.append({
            "output": np.ascontiguousarray(inputs["output"][b0:b0 + BL], dtype=np.float32),
            "context": np.ascontiguousarray(inputs["context"][b0:b0 + BL], dtype=np.float32),
            "attn_weight": np.ascontiguousarray(inputs["attn_weight"], dtype=np.float32),
            "linear_out_w": np.ascontiguousarray(inputs["linear_out_w"], dtype=np.float32),
            "linear_out_b": np.ascontiguousarray(inputs["linear_out_b"], dtype=np.float32),
        })
    return in_maps


LAST_RESULT = None


def kernel(**inputs):
    global LAST_RESULT
    nc = _build()
    in_maps = make_in_maps(inputs)
    trace = os.environ.get("KERNEL_TRACE", "0") == "1"
    res = bass_utils.run_bass_kernel_spmd(
        nc, in_maps, core_ids=list(range(NCORES)), trace=trace)
    LAST_RESULT = res
    out = np.concatenate([r["out"] for r in res.results], axis=0)
    attn = np.concatenate([r["attn"] for r in res.results], axis=0)
    return out, attn


if __name__ == "__main__":
    rng = np.random.default_rng(0)
    inputs = {
        "output": rng.standard_normal((B, T, H), dtype=np.float32),
        "context": rng.standard_normal((B, S, H), dtype=np.float32),
        "attn_weight": (rng.standard_normal((H, H)) / np.sqrt(H)).astype(np.float32),
        "linear_out_w": (rng.standard_normal((H, 2 * H)) / np.sqrt(2 * H)).astype(np.float32),
        "linear_out_b": (rng.standard_normal(H) * 0.01).astype(np.float32),
    }
    out, attn = kernel(**inputs)
    print("out", out.shape, "attn", attn.shape)


# revision 26
# speedup vs baseline: 1.2411x; 1.1199x over previous
"""Trainium2 Bass kernel for a cross-attention nn.Module.

Computes, for inputs (all fp32):
    q      = output @ attn_weight.T              [B,T,H]
    attn   = softmax(q @ context.T, axis=-1)     [B,T,S]
    mix    = attn @ context                      [B,T,H]
    out    = tanh(concat(mix, q) @ linear_out_w.T + linear_out_b)
Returns (out, attn).

Sharding: data-parallel over batch, 2 batches per NeuronCore x 8 cores,
no collectives.  Inside each core:
  - q and scores matmuls run in true fp32 (softmax is extremely sensitive:
    near-one-hot rows; bf16/tf32 scores give absmax errors ~0.02-0.16).
  - mix and linear_out matmuls run as float32r (full PE rate at N>=256).
  - layouts: contraction dims must sit on SBUF partitions, so context is
    PE-transposed once per batch (cT), attn is transposed per tile (attnT),
    mix is computed in natural [t,h] layout then transposed for linear_out.
"""

import os
import sys
import types
import numpy as np
from contextlib import ExitStack

import concourse.bass as bass
import concourse.mybir as mybir
import concourse.tile as tile
from concourse import bass_utils
from concourse.masks import make_identity


def _ensure_ntff_hook():
    """This deployment's antenv package lacks axon_hooks, which
    run_bass_kernel_spmd(trace=True) imports under axon.  Register a shim that
    drives NTFF profiling via ctypes into libaxon_pjrt.so (same contract as
    trn_agent_boot._ntff_profile_via_ctypes)."""
    try:
        from antenv.axon_hooks import get_axon_ntff_profile_hook  # noqa: F401
        return
    except ImportError:
        pass
    import contextlib
    import ctypes

    so_path = "/opt/axon/libaxon_pjrt.so"
    hook = None
    if os.path.exists(so_path):
        lib = ctypes.CDLL(so_path)
        if hasattr(lib, "axon_start_nrt_profile"):
            lib.axon_start_nrt_profile.argtypes = [
                ctypes.POINTER(ctypes.c_int64), ctypes.c_size_t]
            lib.axon_start_nrt_profile.restype = ctypes.c_int64
            lib.axon_stop_nrt_profile.argtypes = [ctypes.c_char_p]
            lib.axon_stop_nrt_profile.restype = ctypes.c_int64

            @contextlib.contextmanager
            def _hook(output_dir, device_ids):
                import jax
                jax.devices()
                if device_ids:
                    ids = (ctypes.c_int64 * len(device_ids))(*device_ids)
                    rc = lib.axon_start_nrt_profile(ids, len(device_ids))
                else:
                    rc = lib.axon_start_nrt_profile(None, 0)
                if rc != 0:
                    raise RuntimeError(f"axon_start_nrt_profile rc={rc}")
                try:
                    yield
                finally:
                    n = lib.axon_stop_nrt_profile(str(output_dir).encode())
                    print(f"ntff profile: {n} file(s) -> {output_dir}")

            hook = _hook

    mod = types.ModuleType("antenv.axon_hooks")
    mod._hook = hook
    mod.get_axon_ntff_profile_hook = lambda: mod._hook
    mod.set_axon_ntff_profile_hook = lambda h: setattr(mod, "_hook", h)
    sys.modules["antenv.axon_hooks"] = mod
    import antenv
    antenv.axon_hooks = mod


_ensure_ntff_hook()

F32 = mybir.dt.float32
F32R = mybir.dt.float32r
BF16 = mybir.dt.bfloat16
AF = mybir.ActivationFunctionType
ALU = mybir.AluOpType
AX = mybir.AxisListType

B, T, S, H = 16, 1024, 2048, 1024
NCORES = 8
BL = B // NCORES            # batches per core
T_CHK = 256                 # t rows per pipeline chunk
N_CHUNKS = T // T_CHK       # 4 per batch
TSUBS = T_CHK // 128        # 2 t-tiles per chunk
HT = H // 128               # 8 feature tiles
ST = S // 128               # 16 context-position tiles
KT2 = 2 * H // 128          # 16 contraction tiles for linear_out


def _emit(ctx: ExitStack, tc: "tile.TileContext", io: dict):
    nc = tc.nc
    a_out, a_attn = io["out"], io["attn"]
    a_output, a_context = io["output"], io["context"]
    a_w, a_w2, a_b2 = io["attn_weight"], io["linear_out_w"], io["linear_out_b"]

    const = ctx.enter_context(tc.tile_pool(name="const", bufs=1))
    ct_pool = ctx.enter_context(tc.tile_pool(name="ct", bufs=1))
    loadc = ctx.enter_context(tc.tile_pool(name="loadc", bufs=2))
    loadout = ctx.enter_context(tc.tile_pool(name="loadout", bufs=1))
    outT_pool = ctx.enter_context(tc.tile_pool(name="outT", bufs=1))
    q_pool = ctx.enter_context(tc.tile_pool(name="q", bufs=2))
    attnT_pool = ctx.enter_context(tc.tile_pool(name="attnT", bufs=1))
    mixnat_pool = ctx.enter_context(tc.tile_pool(name="mixnat", bufs=2))
    mixT_pool = ctx.enter_context(tc.tile_pool(name="mixT", bufs=1))
    astage = ctx.enter_context(tc.tile_pool(name="astage", bufs=2))
    ostage = ctx.enter_context(tc.tile_pool(name="ostage", bufs=1))
    w2s_pool = ctx.enter_context(tc.tile_pool(name="w2s", bufs=2))
    small = ctx.enter_context(tc.tile_pool(name="small", bufs=3))
    dram = ctx.enter_context(tc.tile_pool(name="dram", bufs=1, space="DRAM"))
    ps_s = ctx.enter_context(tc.tile_pool(name="ps_s", bufs=2, space="PSUM"))
    ps_long = ctx.enter_context(tc.tile_pool(name="ps_long", bufs=4, space="PSUM"))
    ps_mm = ctx.enter_context(tc.tile_pool(name="ps_mm", bufs=2, space="PSUM"))

    def transpose_to_psum(src_ap, n, ident):
        assert n <= 4
        pst_flat = ps_mm.tile([128, 512], F32, tag="mm", name="pst")
        pst = pst_flat.rearrange("p (a c) -> p a c", a=4)
        for i in range(n):
            nc.tensor.transpose(pst[:, i, :], src_ap[:, i * 128:(i + 1) * 128], ident)
        return pst

    def transpose_pack(src_ap, n, dst_ap, ident):
        pst = transpose_to_psum(src_ap, n, ident)
        nc.vector.tensor_copy(dst_ap, pst[:, :n, :])

    def transpose_split(src_ap, n, dst_hi, dst_lo, ident):
        pst = transpose_to_psum(src_ap, n, ident)
        nc.scalar.copy(dst_hi, pst[:, :n, :])
        nc.vector.tensor_tensor(dst_lo, pst[:, :n, :], dst_hi, op=ALU.subtract)

    # ---------------- phase 0: constants ----------------
    ident = const.tile([128, 128], F32)
    make_identity(nc, ident)

    b2bc = const.tile([128, H], F32)
    nc.gpsimd.dma_start(out=b2bc, in_=a_b2.partition_broadcast(128))

    WTh = const.tile([128, HT, H], BF16)
    WTl = const.tile([128, HT, H], BF16)
    for og in range(HT):
        wl = astage.tile([128, 2 * H], F32, tag="astage")
        nc.sync.dma_start(out=wl[:, :H], in_=a_w[og * 128:(og + 1) * 128, :])
        for hg in range(2):
            s1 = slice(hg * 4, (hg + 1) * 4)
            s2 = slice(og * 128, (og + 1) * 128)
            transpose_split(wl[:, hg * 512:(hg + 1) * 512], 4,
                            WTh[:, s1, s2], WTl[:, s1, s2], ident)

    w2t = dram.tile([2 * H, H], F32R)
    for og in range(HT):
        w2l = astage.tile([128, 2 * H], F32, tag="astage")
        nc.sync.dma_start(out=w2l, in_=a_w2[og * 128:(og + 1) * 128, :])
        for kg in range(4):
            stg = mixnat_pool.tile([128, 4, 128], F32R, tag="mixnat")
            pst = transpose_to_psum(w2l[:, kg * 512:(kg + 1) * 512], 4, ident)
            nc.vector.tensor_copy(stg, pst)
            dst = w2t[kg * 512:(kg + 1) * 512, og * 128:(og + 1) * 128]
            nc.sync.dma_start(out=dst.rearrange("(a p) o -> p a o", p=128), in_=stg)

    # ---------------- pipelined chunk jobs ----------------
    # Per iteration j we interleave (thunk-by-thunk) the emission of:
    #   - scores+softmax of job j          (DMA-free, dense PE work)
    #   - the "tail" of job j-1            (attnT transposes, mix, mixT, lin --
    #                                       DMA-hungry, latency-bound)
    # so the PE never drains while cm/w2s streams catch up, and the DMA
    # bursts spread over the whole job span.  Then A/B (load+transpose
    # output, q matmuls) for job j+1.
    jobs = [(b, ck) for b in range(BL) for ck in range(N_CHUNKS)]
    cT = {}
    qbuf = {}
    astash = {}

    def scores_thunks(idx):
        """Dep-closed emission thunks for scores+softmax of job idx.
        sc-outer: each psum tile fully accumulates then evacuates within
        one thunk, so interleaved streams can never invert engine orders."""
        b, ck = jobs[idx]
        t0 = ck * T_CHK
        q_hi, q_lo = qbuf[idx]
        cTh, cTl = cT[b]
        state = {}

        def sc_block(tsub, sc):
            tsl = slice(tsub * 128, (tsub + 1) * 128)
            pss = ps_s.tile([128, 512], F32, tag="ps", name=f"pss{idx}_{tsub}_{sc}")
            passes = [(q_hi, cTh), (q_hi, cTl), (q_lo, cTh)]
            for kt in range(HT):
                for pi, (qsrc, csrc) in enumerate(passes):
                    nc.tensor.matmul(
                        pss,
                        qsrc[:, kt, tsl],
                        csrc[:, kt, sc * 512:(sc + 1) * 512],
                        start=(kt == 0 and pi == 0),
                        stop=(kt == HT - 1 and pi == 2))
            ast = state[tsub]
            if sc % 2 == 0:
                nc.vector.tensor_copy(ast[:, sc * 512:(sc + 1) * 512], pss)
            else:
                nc.scalar.copy(ast[:, sc * 512:(sc + 1) * 512], pss)

        def softmax(tsub):
            r0 = t0 + tsub * 128
            ast = state[tsub]
            stats = small.tile([128, 4], F32, tag="sm")
            nm, lsum, rinv = stats[:, 0:1], stats[:, 1:2], stats[:, 2:3]
            nc.vector.tensor_reduce(nm, ast, axis=AX.X, op=ALU.max, negate=True)
            nc.scalar.activation(ast, ast, AF.Exp, bias=nm, scale=1.0, accum_out=lsum)
            nc.vector.reciprocal(rinv, lsum)
            nc.vector.tensor_scalar_mul(ast, ast, rinv)
            nc.sync.dma_start(out=a_attn[b, r0:r0 + 128, :], in_=ast)
            astash.setdefault(idx, {})[tsub] = ast

        for tsub in range(TSUBS):
            def ast_alloc(ts=tsub):
                state[ts] = astage.tile([128, S], F32, tag="astage",
                                        name=f"ast{idx}_{ts}")
            yield ast_alloc
            for sc in range(4):
                yield (lambda ts=tsub, s=sc: sc_block(ts, s))
            yield (lambda ts=tsub: softmax(ts))

    def ab_thunks(idx):
        """Dep-closed thunks for output-load/transpose + q of job idx."""
        b, ck = jobs[idx]
        t0 = ck * T_CHK
        state = {}

        def alloc():
            state['oTh'] = outT_pool.tile([128, HT, T_CHK], BF16, tag="oth",
                                          name=f"oth{idx}")
            state['oTl'] = outT_pool.tile([128, HT, T_CHK], BF16, tag="otl",
                                          name=f"otl{idx}")
            state['q_hi'] = q_pool.tile([128, HT, T_CHK], BF16, tag="qhi",
                                        name=f"qhi{idx}", bufs=2)
            state['q_lo'] = q_pool.tile([128, HT, T_CHK], BF16, tag="qlo",
                                        name=f"qlo{idx}", bufs=2)
            qbuf[idx] = (state['q_hi'], state['q_lo'])

        def load_tr(tsub):
            ld = loadout.tile([128, H], F32, tag="oload")
            nc.sync.dma_start(
                out=ld, in_=a_output[b, t0 + tsub * 128:t0 + (tsub + 1) * 128, :])
            for hg in range(2):
                s1 = slice(hg * 4, (hg + 1) * 4)
                s2 = slice(tsub * 128, (tsub + 1) * 128)
                transpose_split(ld[:, hg * 512:(hg + 1) * 512], 4,
                                state['oTh'][:, s1, s2], state['oTl'][:, s1, s2],
                                ident)

        def q_ot(ot):
            psq = ps_mm.tile([128, 512], F32, tag="mm")
            tgt = psq[:, :T_CHK]
            for kt in range(HT):
                passes = [(state['oTh'], 0), (state['oTl'], 0), (state['oTh'], 1)]
                srcs = [(WTh, state['oTh']), (WTh, state['oTl']), (WTl, state['oTh'])]
                for pi, (wsrc, osrc) in enumerate(srcs):
                    nc.tensor.matmul(
                        tgt,
                        wsrc[:, kt, ot * 128:(ot + 1) * 128],
                        osrc[:, kt, :],
                        start=(kt == 0 and pi == 0),
                        stop=(kt == HT - 1 and pi == 2))
            osl = slice(ot, ot + 1)
            view = tgt.rearrange("p (a c) -> p a c", a=1)
            nc.scalar.copy(state['q_hi'][:, osl, :], view)
            nc.vector.tensor_tensor(state['q_lo'][:, osl, :], view,
                                    state['q_hi'][:, osl, :], op=ALU.subtract)

        yield alloc
        for tsub in range(TSUBS):
            yield (lambda ts=tsub: load_tr(ts))
        for ot in range(HT):
            yield (lambda o=ot: q_ot(o))

    def ct_thunks(b):
        state = {}

        def alloc():
            state['h'] = ct_pool.tile([128, HT, S], BF16, tag="cth", name=f"cth{b}")
            state['l'] = ct_pool.tile([128, HT, S], BF16, tag="ctl", name=f"ctl{b}")
            cT[b] = (state['h'], state['l'])

        def one(st):
            cl = loadc.tile([128, H], F32, tag="cload")
            nc.sync.dma_start(out=cl, in_=a_context[b, st * 128:(st + 1) * 128, :])
            for hg in range(2):
                s1, s2 = slice(hg * 4, (hg + 1) * 4), slice(st * 128, (st + 1) * 128)
                transpose_split(cl[:, hg * 512:(hg + 1) * 512], 4,
                                state['h'][:, s1, s2], state['l'][:, s1, s2], ident)

        yield alloc
        for st in range(ST):
            yield (lambda s=st: one(s))

    def tail_thunks(idx):
        """attnT/qr prologue (uninterleaved) then mix/mixT/lin thunks.
        Every pool alloc only waits on tiles whose readers were emitted
        earlier, so interleaving cannot create cross-engine wait cycles."""
        b, ck = jobs[idx]
        t0 = ck * T_CHK
        state = {}

        def attnt_alloc():
            state['attnT'] = attnT_pool.tile([128, ST, T_CHK], F32R, tag="attnT",
                                             name=f"attnT{idx}")

        def attnt_tr(tsub, sg):
            asts = astash[idx]
            tsl = slice(tsub * 128, (tsub + 1) * 128)
            transpose_pack(asts[tsub][:, sg * 512:(sg + 1) * 512], 4,
                           state['attnT'][:, sg * 4:(sg + 1) * 4, tsl], ident)

        def qr_build():
            q_hi, q_lo = qbuf[idx]
            qr = q_pool.tile([128, HT, T_CHK], F32R, tag="qr",
                             name=f"qr{idx}", bufs=1)
            nc.vector.tensor_tensor(qr, q_hi, q_lo, op=ALU.add)
            state['q_r'] = qr

        def mix_alloc():
            state['psm'] = [ps_long.tile([128, 512], F32, tag="ml",
                                         name=f"psm{idx}_{i}")
                            for i in range(TSUBS * 2)]

        def mix_st(st):
            cm = loadc.tile([128, H], F32R, tag="cload")
            nc.gpsimd.dma_start(out=cm, in_=a_context[b, st * 128:(st + 1) * 128, :])
            for tsub in range(TSUBS):
                lhsT = state['attnT'][:, st, tsub * 128:(tsub + 1) * 128]
                for nchk in range(2):
                    nc.tensor.matmul(
                        state['psm'][tsub * 2 + nchk],
                        lhsT,
                        cm[:, nchk * 512:(nchk + 1) * 512],
                        start=(st == 0), stop=(st == ST - 1))

        def mn_copy(tsub):
            mn = mixnat_pool.tile([128, H], F32, tag="mixnat",
                                  name=f"mn{idx}_{tsub}")
            nc.vector.tensor_copy(mn[:, :512], state['psm'][tsub * 2])
            nc.scalar.copy(mn[:, 512:], state['psm'][tsub * 2 + 1])
            state.setdefault('mns', {})[tsub] = mn

        def mixt_alloc():
            state['mixT'] = mixT_pool.tile([128, HT, T_CHK], F32R, tag="mixT",
                                           name=f"mixT{idx}")

        def mixt_tr(tsub, hg):
            transpose_pack(state['mns'][tsub][:, hg * 512:(hg + 1) * 512], 4,
                           state['mixT'][:, hg * 4:(hg + 1) * 4,
                                         tsub * 128:(tsub + 1) * 128], ident)

        def lin_alloc():
            state['pso'] = [ps_long.tile([128, 512], F32, tag="ml",
                                         name=f"pso{idx}_{i}")
                            for i in range(TSUBS * 2)]

        def lin_kt(kt):
            w2s = w2s_pool.tile([128, H], F32R, tag="w2s")
            nc.sync.dma_start(out=w2s, in_=w2t[kt * 128:(kt + 1) * 128, :])
            for tsub in range(TSUBS):
                if kt < HT:
                    lhsT = state['mixT'][:, kt, tsub * 128:(tsub + 1) * 128]
                else:
                    lhsT = state['q_r'][:, kt - HT, tsub * 128:(tsub + 1) * 128]
                for nchk in range(2):
                    nc.tensor.matmul(
                        state['pso'][tsub * 2 + nchk],
                        lhsT,
                        w2s[:, nchk * 512:(nchk + 1) * 512],
                        start=(kt == 0), stop=(kt == KT2 - 1))

        def finish(tsub):
            ost = ostage.tile([128, H], F32, tag="ostage")
            for nchk in range(2):
                nc.vector.tensor_tensor(state['pso'][tsub * 2 + nchk],
                                        state['pso'][tsub * 2 + nchk],
                                        b2bc[:, nchk * 512:(nchk + 1) * 512],
                                        op=ALU.add)
                nc.scalar.activation(ost[:, nchk * 512:(nchk + 1) * 512],
                                     state['pso'][tsub * 2 + nchk], AF.Tanh)
            r0 = t0 + tsub * 128
            nc.sync.dma_start(out=a_out[b, r0:r0 + 128, :], in_=ost)

        prologue = [attnt_alloc]
        for tsub in range(TSUBS):
            for sg in range(4):
                prologue.append(lambda ts=tsub, s=sg: attnt_tr(ts, s))
        prologue.append(qr_build)

        rest = [mix_alloc]
        for st in range(ST):
            rest.append(lambda s=st: mix_st(s))
        for tsub in range(TSUBS):
            rest.append(lambda ts=tsub: mn_copy(ts))
        rest.append(mixt_alloc)
        for tsub in range(TSUBS):
            for hg in range(2):
                rest.append(lambda ts=tsub, h=hg: mixt_tr(ts, h))
        rest.append(lin_alloc)
        for kt in range(KT2):
            rest.append(lambda k=kt: lin_kt(k))
        for tsub in range(TSUBS):
            rest.append(lambda ts=tsub: finish(ts))
        return prologue, rest

    import itertools

    def chain(*gens):
        return itertools.chain(*[g for g in gens if g is not None])

    # prologue: context(0) + A/B(0)
    for th in chain(ct_thunks(0), ab_thunks(0)):
        th()
    for idx, (b, ck) in enumerate(jobs):
        primary = [scores_thunks(idx)]
        if idx + 1 < len(jobs):
            nb, nck = jobs[idx + 1]
            primary.append(ab_thunks(idx + 1))
            if nb != b:
                primary.append(ct_thunks(nb))
        p = list(chain(*primary))
        if idx > 0:
            pro, rest = tail_thunks(idx - 1)
            for th in pro:
                th()
        else:
            rest = []
        # proportional interleave: spread `rest` across `p`
        np_, ns = len(p), len(rest)
        si = 0
        for k, th in enumerate(p):
            th()
            target = (k + 1) * ns // np_ if np_ else ns
            while si < target:
                rest[si]()
                si += 1
        while si < ns:
            rest[si]()
            si += 1
    pro, rest = tail_thunks(len(jobs) - 1)
    for th in pro + rest:
        th()


def _split_sync_waits(nc):
    """This walrus/ISA build accepts at most ONE sync-wait command per
    instruction, but Tile's sem-assigner can attach several (phase-first
    instructions, kernel-tail drain).  Split: keep the last wait on the
    instruction, hoist the rest onto same-engine NoOps inserted just before."""
    n_split = 0
    for fn in nc.m.functions:
        for blk in fn.blocks:
            insts = blk.instructions
            out = []
            changed = False
            for inst in insts:
                si = inst.sync_info
                waits = list(si.on_wait) if (si and si.on_wait) else []
                if len(waits) > 1:
                    for w in waits[:-1]:
                        nop = mybir.InstNoOp(
                            name=f"waitsplit-{nc.next_id()}",
                            engine=inst.engine,
                            sync_info=mybir.SyncInfo(on_wait=[w], on_update=[]),
                        )
                        out.append(nop)
                    inst.sync_info = mybir.SyncInfo(
                        on_wait=[waits[-1]], on_update=list(si.on_update or []))
                    n_split += 1
                    changed = True
                out.append(inst)
            if changed:
                blk.instructions = out
    return n_split


_CACHED_NC = {}


def _build(split_waits=True):
    if split_waits in _CACHED_NC:
        return _CACHED_NC[split_waits]
    nc = bass.Bass("TRN2", target_bir_lowering=False, debug=False)
    io = {
        "output": nc.dram_tensor("output", [BL, T, H], F32, kind="ExternalInput").ap(),
        "context": nc.dram_tensor("context", [BL, S, H], F32, kind="ExternalInput").ap(),
        "attn_weight": nc.dram_tensor("attn_weight", [H, H], F32, kind="ExternalInput").ap(),
        "linear_out_w": nc.dram_tensor("linear_out_w", [H, 2 * H], F32, kind="ExternalInput").ap(),
        "linear_out_b": nc.dram_tensor("linear_out_b", [H], F32, kind="ExternalInput").ap(),
        "out": nc.dram_tensor("out", [BL, T, H], F32, kind="ExternalOutput").ap(),
        "attn": nc.dram_tensor("attn", [BL, T, S], F32, kind="ExternalOutput").ap(),
    }
    with tile.TileContext(nc) as tc:
        with ExitStack() as ctx:
            _emit(ctx, tc, io)
    if split_waits:
        # CoreSim can't execute the bare NoOps; only split for the HW path.
        _split_sync_waits(nc)
    _CACHED_NC[split_waits] = nc
    return nc


def make_in_maps(inputs):
    in_maps = []
    for c in range(NCORES):
        b0 = c * BL
        in_maps.append({
            "output": np.ascontiguousarray(inputs["output"][b0:b0 + BL], dtype=np.float32),
            "context": np.ascontiguousarray(inputs["context"][b0:b0 + BL], dtype=np.float32),
            "attn_weight": np.ascontiguousarray(inputs["attn_weight"], dtype=np.float32),
            "linear_out_w": np.ascontiguousarray(inputs["linear_out_w"], dtype=np.float32),
            "linear_out_b": np.ascontiguousarray(inputs["linear_out_b"], dtype=np.float32),
        })
    return in_maps


LAST_RESULT = None


def kernel(**inputs):
    global LAST_RESULT
    nc = _build()
    in_maps = make_in_maps(inputs)
    trace = os.environ.get("KERNEL_TRACE", "0") == "1"
    res = bass_utils.run_bass_kernel_spmd(
        nc, in_maps, core_ids=list(range(NCORES)), trace=trace)
    LAST_RESULT = res
    out = np.concatenate([r["out"] for r in res.results], axis=0)
    attn = np.concatenate([r["attn"] for r in res.results], axis=0)
    return out, attn


if __name__ == "__main__":
    rng = np.random.default_rng(0)
    inputs = {
        "output": rng.standard_normal((B, T, H), dtype=np.float32),
        "context": rng.standard_normal((B, S, H), dtype=np.float32),
        "attn_weight": (rng.standard_normal((H, H)) / np.sqrt(H)).astype(np.float32),
        "linear_out_w": (rng.standard_normal((H, 2 * H)) / np.sqrt(2 * H)).astype(np.float32),
        "linear_out_b": (rng.standard_normal(H) * 0.01).astype(np.float32),
    }
    out, attn = kernel(**inputs)
    print("out", out.shape, "attn", attn.shape)


# revision 27
# speedup vs baseline: 1.3841x; 1.1153x over previous
"""Trainium2 Bass kernel for a cross-attention nn.Module.

Computes, for inputs (all fp32):
    q      = output @ attn_weight.T              [B,T,H]
    attn   = softmax(q @ context.T, axis=-1)     [B,T,S]
    mix    = attn @ context                      [B,T,H]
    out    = tanh(concat(mix, q) @ linear_out_w.T + linear_out_b)
Returns (out, attn).

Sharding: data-parallel over batch, 2 batches per NeuronCore x 8 cores,
no collectives.  Inside each core:
  - q and scores matmuls run in true fp32 (softmax is extremely sensitive:
    near-one-hot rows; bf16/tf32 scores give absmax errors ~0.02-0.16).
  - mix and linear_out matmuls run as float32r (full PE rate at N>=256).
  - layouts: contraction dims must sit on SBUF partitions, so context is
    PE-transposed once per batch (cT), attn is transposed per tile (attnT),
    mix is computed in natural [t,h] layout then transposed for linear_out.
"""

import os
import sys
import types
import numpy as np
from contextlib import ExitStack

import concourse.bass as bass
import concourse.mybir as mybir
import concourse.tile as tile
from concourse import bass_utils
from concourse.masks import make_identity


def _ensure_ntff_hook():
    """This deployment's antenv package lacks axon_hooks, which
    run_bass_kernel_spmd(trace=True) imports under axon.  Register a shim that
    drives NTFF profiling via ctypes into libaxon_pjrt.so (same contract as
    trn_agent_boot._ntff_profile_via_ctypes)."""
    try:
        from antenv.axon_hooks import get_axon_ntff_profile_hook  # noqa: F401
        return
    except ImportError:
        pass
    import contextlib
    import ctypes

    so_path = "/opt/axon/libaxon_pjrt.so"
    hook = None
    if os.path.exists(so_path):
        lib = ctypes.CDLL(so_path)
        if hasattr(lib, "axon_start_nrt_profile"):
            lib.axon_start_nrt_profile.argtypes = [
                ctypes.POINTER(ctypes.c_int64), ctypes.c_size_t]
            lib.axon_start_nrt_profile.restype = ctypes.c_int64
            lib.axon_stop_nrt_profile.argtypes = [ctypes.c_char_p]
            lib.axon_stop_nrt_profile.restype = ctypes.c_int64

            @contextlib.contextmanager
            def _hook(output_dir, device_ids):
                import jax
                jax.devices()
                if device_ids:
                    ids = (ctypes.c_int64 * len(device_ids))(*device_ids)
                    rc = lib.axon_start_nrt_profile(ids, len(device_ids))
                else:
                    rc = lib.axon_start_nrt_profile(None, 0)
                if rc != 0:
                    raise RuntimeError(f"axon_start_nrt_profile rc={rc}")
                try:
                    yield
                finally:
                    n = lib.axon_stop_nrt_profile(str(output_dir).encode())
                    print(f"ntff profile: {n} file(s) -> {output_dir}")

            hook = _hook

    mod = types.ModuleType("antenv.axon_hooks")
    mod._hook = hook
    mod.get_axon_ntff_profile_hook = lambda: mod._hook
    mod.set_axon_ntff_profile_hook = lambda h: setattr(mod, "_hook", h)
    sys.modules["antenv.axon_hooks"] = mod
    import antenv
    antenv.axon_hooks = mod


_ensure_ntff_hook()

F32 = mybir.dt.float32
F32R = mybir.dt.float32r
BF16 = mybir.dt.bfloat16
AF = mybir.ActivationFunctionType
ALU = mybir.AluOpType
AX = mybir.AxisListType

B, T, S, H = 16, 1024, 2048, 1024
NCORES = 8
BL = B // NCORES            # batches per core
T_CHK = 256                 # t rows per pipeline chunk
N_CHUNKS = T // T_CHK       # 4 per batch
TSUBS = T_CHK // 128        # 2 t-tiles per chunk
HT = H // 128               # 8 feature tiles
ST = S // 128               # 16 context-position tiles
KT2 = 2 * H // 128          # 16 contraction tiles for linear_out


def _emit(ctx: ExitStack, tc: "tile.TileContext", io: dict):
    nc = tc.nc
    a_out, a_attn = io["out"], io["attn"]
    a_output, a_context = io["output"], io["context"]
    a_w, a_w2, a_b2 = io["attn_weight"], io["linear_out_w"], io["linear_out_b"]

    const = ctx.enter_context(tc.tile_pool(name="const", bufs=1))
    ct_pool = ctx.enter_context(tc.tile_pool(name="ct", bufs=1))
    loadc = ctx.enter_context(tc.tile_pool(name="loadc", bufs=2))
    loadout = ctx.enter_context(tc.tile_pool(name="loadout", bufs=1))
    outT_pool = ctx.enter_context(tc.tile_pool(name="outT", bufs=1))
    q_pool = ctx.enter_context(tc.tile_pool(name="q", bufs=2))
    attnT_pool = ctx.enter_context(tc.tile_pool(name="attnT", bufs=1))
    mixnat_pool = ctx.enter_context(tc.tile_pool(name="mixnat", bufs=2))
    mixT_pool = ctx.enter_context(tc.tile_pool(name="mixT", bufs=1))
    astage = ctx.enter_context(tc.tile_pool(name="astage", bufs=2))
    ostage = ctx.enter_context(tc.tile_pool(name="ostage", bufs=1))
    w2s_pool = ctx.enter_context(tc.tile_pool(name="w2s", bufs=2))
    small = ctx.enter_context(tc.tile_pool(name="small", bufs=3))
    dram = ctx.enter_context(tc.tile_pool(name="dram", bufs=1, space="DRAM"))
    ps_s = ctx.enter_context(tc.tile_pool(name="ps_s", bufs=2, space="PSUM"))
    ps_long = ctx.enter_context(tc.tile_pool(name="ps_long", bufs=4, space="PSUM"))
    ps_mm = ctx.enter_context(tc.tile_pool(name="ps_mm", bufs=2, space="PSUM"))

    def transpose_to_psum(src_ap, n, ident):
        assert n <= 4
        pst_flat = ps_mm.tile([128, 512], F32, tag="mm", name="pst")
        pst = pst_flat.rearrange("p (a c) -> p a c", a=4)
        for i in range(n):
            nc.tensor.transpose(pst[:, i, :], src_ap[:, i * 128:(i + 1) * 128], ident)
        return pst

    def transpose_pack(src_ap, n, dst_ap, ident, on_act=False):
        pst = transpose_to_psum(src_ap, n, ident)
        if on_act:
            nc.scalar.copy(dst_ap, pst[:, :n, :])
        else:
            nc.vector.tensor_copy(dst_ap, pst[:, :n, :])

    def transpose_split(src_ap, n, dst_hi, dst_lo, ident):
        pst = transpose_to_psum(src_ap, n, ident)
        nc.scalar.copy(dst_hi, pst[:, :n, :])
        nc.vector.tensor_tensor(dst_lo, pst[:, :n, :], dst_hi, op=ALU.subtract)

    # ---------------- phase 0: constants ----------------
    ident = const.tile([128, 128], F32)
    make_identity(nc, ident)

    b2bc = const.tile([128, H], F32)
    nc.gpsimd.dma_start(out=b2bc, in_=a_b2.partition_broadcast(128))

    WTh = const.tile([128, HT, H], BF16)
    WTl = const.tile([128, HT, H], BF16)
    w2t = dram.tile([2 * H, H], F32R)

    def weight_thunks():
        def wt_og(og):
            wl = astage.tile([128, 2 * H], F32, tag="astage")
            nc.sync.dma_start(out=wl[:, :H], in_=a_w[og * 128:(og + 1) * 128, :])
            for hg in range(2):
                s1 = slice(hg * 4, (hg + 1) * 4)
                s2 = slice(og * 128, (og + 1) * 128)
                transpose_split(wl[:, hg * 512:(hg + 1) * 512], 4,
                                WTh[:, s1, s2], WTl[:, s1, s2], ident)

        def w2t_og(og):
            w2l = astage.tile([128, 2 * H], F32, tag="astage")
            nc.sync.dma_start(out=w2l, in_=a_w2[og * 128:(og + 1) * 128, :])
            for kg in range(4):
                stg = mixnat_pool.tile([128, 4, 128], F32R, tag="mixnat")
                pst = transpose_to_psum(w2l[:, kg * 512:(kg + 1) * 512], 4, ident)
                nc.vector.tensor_copy(stg, pst)
                dst = w2t[kg * 512:(kg + 1) * 512, og * 128:(og + 1) * 128]
                nc.sync.dma_start(out=dst.rearrange("(a p) o -> p a o", p=128), in_=stg)

        for og in range(HT):
            yield (lambda o=og: wt_og(o))
        for og in range(HT):
            yield (lambda o=og: w2t_og(o))

    # ---------------- pipelined chunk jobs ----------------
    # Per iteration j we interleave (thunk-by-thunk) the emission of:
    #   - scores+softmax of job j          (DMA-free, dense PE work)
    #   - the "tail" of job j-1            (attnT transposes, mix, mixT, lin --
    #                                       DMA-hungry, latency-bound)
    # so the PE never drains while cm/w2s streams catch up, and the DMA
    # bursts spread over the whole job span.  Then A/B (load+transpose
    # output, q matmuls) for job j+1.
    jobs = [(b, ck) for b in range(BL) for ck in range(N_CHUNKS)]
    cT = {}
    qbuf = {}
    astash = {}

    def scores_thunks(idx):
        """Dep-closed emission thunks for scores+softmax of job idx.
        sc-outer: each psum tile fully accumulates then evacuates within
        one thunk, so interleaved streams can never invert engine orders."""
        b, ck = jobs[idx]
        t0 = ck * T_CHK
        q_hi, q_lo = qbuf[idx]
        cTh, cTl = cT[b]
        state = {}

        def sc_block(tsub, sc):
            tsl = slice(tsub * 128, (tsub + 1) * 128)
            pss = ps_s.tile([128, 512], F32, tag="ps", name=f"pss{idx}_{tsub}_{sc}")
            passes = [(q_hi, cTh), (q_hi, cTl), (q_lo, cTh)]
            for kt in range(HT):
                for pi, (qsrc, csrc) in enumerate(passes):
                    nc.tensor.matmul(
                        pss,
                        qsrc[:, kt, tsl],
                        csrc[:, kt, sc * 512:(sc + 1) * 512],
                        start=(kt == 0 and pi == 0),
                        stop=(kt == HT - 1 and pi == 2))
            ast = state[tsub]
            if sc % 2 == 0:
                nc.vector.tensor_copy(ast[:, sc * 512:(sc + 1) * 512], pss)
            else:
                nc.scalar.copy(ast[:, sc * 512:(sc + 1) * 512], pss)

        def softmax(tsub):
            r0 = t0 + tsub * 128
            ast = state[tsub]
            stats = small.tile([128, 4], F32, tag="sm")
            nm, lsum, rinv = stats[:, 0:1], stats[:, 1:2], stats[:, 2:3]
            nc.vector.tensor_reduce(nm, ast, axis=AX.X, op=ALU.max, negate=True)
            nc.scalar.activation(ast, ast, AF.Exp, bias=nm, scale=1.0, accum_out=lsum)
            nc.vector.reciprocal(rinv, lsum)
            nc.vector.tensor_scalar_mul(ast, ast, rinv)
            nc.sync.dma_start(out=a_attn[b, r0:r0 + 128, :], in_=ast)
            astash.setdefault(idx, {})[tsub] = ast

        for tsub in range(TSUBS):
            def ast_alloc(ts=tsub):
                state[ts] = astage.tile([128, S], F32, tag="astage",
                                        name=f"ast{idx}_{ts}")
            yield ast_alloc
            for sc in range(4):
                yield (lambda ts=tsub, s=sc: sc_block(ts, s))
            yield (lambda ts=tsub: softmax(ts))

    def ab_thunks(idx):
        """Dep-closed thunks for output-load/transpose + q of job idx."""
        b, ck = jobs[idx]
        t0 = ck * T_CHK
        state = {}

        def alloc():
            state['oTh'] = outT_pool.tile([128, HT, T_CHK], BF16, tag="oth",
                                          name=f"oth{idx}")
            state['oTl'] = outT_pool.tile([128, HT, T_CHK], BF16, tag="otl",
                                          name=f"otl{idx}")
            state['q_hi'] = q_pool.tile([128, HT, T_CHK], BF16, tag="qhi",
                                        name=f"qhi{idx}", bufs=2)
            state['q_lo'] = q_pool.tile([128, HT, T_CHK], BF16, tag="qlo",
                                        name=f"qlo{idx}", bufs=2)
            qbuf[idx] = (state['q_hi'], state['q_lo'])

        def load_tr(tsub):
            ld = loadout.tile([128, H], F32, tag="oload")
            nc.sync.dma_start(
                out=ld, in_=a_output[b, t0 + tsub * 128:t0 + (tsub + 1) * 128, :])
            for hg in range(2):
                s1 = slice(hg * 4, (hg + 1) * 4)
                s2 = slice(tsub * 128, (tsub + 1) * 128)
                transpose_split(ld[:, hg * 512:(hg + 1) * 512], 4,
                                state['oTh'][:, s1, s2], state['oTl'][:, s1, s2],
                                ident)

        def q_ot(ot):
            psq = ps_mm.tile([128, 512], F32, tag="mm")
            tgt = psq[:, :T_CHK]
            for kt in range(HT):
                passes = [(state['oTh'], 0), (state['oTl'], 0), (state['oTh'], 1)]
                srcs = [(WTh, state['oTh']), (WTh, state['oTl']), (WTl, state['oTh'])]
                for pi, (wsrc, osrc) in enumerate(srcs):
                    nc.tensor.matmul(
                        tgt,
                        wsrc[:, kt, ot * 128:(ot + 1) * 128],
                        osrc[:, kt, :],
                        start=(kt == 0 and pi == 0),
                        stop=(kt == HT - 1 and pi == 2))
            osl = slice(ot, ot + 1)
            view = tgt.rearrange("p (a c) -> p a c", a=1)
            nc.scalar.copy(state['q_hi'][:, osl, :], view)
            nc.vector.tensor_tensor(state['q_lo'][:, osl, :], view,
                                    state['q_hi'][:, osl, :], op=ALU.subtract)

        yield alloc
        for tsub in range(TSUBS):
            yield (lambda ts=tsub: load_tr(ts))
        for ot in range(HT):
            yield (lambda o=ot: q_ot(o))

    def ct_thunks(b):
        state = {}

        def alloc():
            state['h'] = ct_pool.tile([128, HT, S], BF16, tag="cth", name=f"cth{b}")
            state['l'] = ct_pool.tile([128, HT, S], BF16, tag="ctl", name=f"ctl{b}")
            cT[b] = (state['h'], state['l'])

        def one(st):
            cl = loadc.tile([128, H], F32, tag="cload")
            nc.sync.dma_start(out=cl, in_=a_context[b, st * 128:(st + 1) * 128, :])
            for hg in range(2):
                s1, s2 = slice(hg * 4, (hg + 1) * 4), slice(st * 128, (st + 1) * 128)
                transpose_split(cl[:, hg * 512:(hg + 1) * 512], 4,
                                state['h'][:, s1, s2], state['l'][:, s1, s2], ident)

        yield alloc
        for st in range(ST):
            yield (lambda s=st: one(s))

    def tail_thunks(idx):
        """attnT/qr prologue (uninterleaved) then mix/mixT/lin thunks.
        Every pool alloc only waits on tiles whose readers were emitted
        earlier, so interleaving cannot create cross-engine wait cycles."""
        b, ck = jobs[idx]
        t0 = ck * T_CHK
        state = {}

        def attnt_alloc():
            state['attnT'] = attnT_pool.tile([128, ST, T_CHK], F32R, tag="attnT",
                                             name=f"attnT{idx}")

        def attnt_tr(tsub, sg):
            asts = astash[idx]
            tsl = slice(tsub * 128, (tsub + 1) * 128)
            transpose_pack(asts[tsub][:, sg * 512:(sg + 1) * 512], 4,
                           state['attnT'][:, sg * 4:(sg + 1) * 4, tsl], ident,
                           on_act=(sg % 2 == 1))

        def qr_build():
            q_hi, q_lo = qbuf[idx]
            qr = q_pool.tile([128, HT, T_CHK], F32R, tag="qr",
                             name=f"qr{idx}", bufs=1)
            nc.vector.tensor_tensor(qr, q_hi, q_lo, op=ALU.add)
            state['q_r'] = qr

        def mix_alloc():
            state['psm'] = [ps_long.tile([128, 512], F32, tag="ml",
                                         name=f"psm{idx}_{i}")
                            for i in range(TSUBS * 2)]

        def mix_st(st):
            cm = loadc.tile([128, H], F32R, tag="cload")
            nc.gpsimd.dma_start(out=cm, in_=a_context[b, st * 128:(st + 1) * 128, :])
            for tsub in range(TSUBS):
                lhsT = state['attnT'][:, st, tsub * 128:(tsub + 1) * 128]
                for nchk in range(2):
                    nc.tensor.matmul(
                        state['psm'][tsub * 2 + nchk],
                        lhsT,
                        cm[:, nchk * 512:(nchk + 1) * 512],
                        start=(st == 0), stop=(st == ST - 1))

        def mn_copy(tsub):
            mn = mixnat_pool.tile([128, H], F32, tag="mixnat",
                                  name=f"mn{idx}_{tsub}")
            nc.vector.tensor_copy(mn[:, :512], state['psm'][tsub * 2])
            nc.scalar.copy(mn[:, 512:], state['psm'][tsub * 2 + 1])
            state.setdefault('mns', {})[tsub] = mn

        def mixt_alloc():
            state['mixT'] = mixT_pool.tile([128, HT, T_CHK], F32R, tag="mixT",
                                           name=f"mixT{idx}")

        def mixt_tr(tsub, hg):
            transpose_pack(state['mns'][tsub][:, hg * 512:(hg + 1) * 512], 4,
                           state['mixT'][:, hg * 4:(hg + 1) * 4,
                                         tsub * 128:(tsub + 1) * 128], ident,
                           on_act=(hg == 1))

        def lin_alloc():
            state['pso'] = [ps_long.tile([128, 512], F32, tag="ml",
                                         name=f"pso{idx}_{i}")
                            for i in range(TSUBS * 2)]

        def lin_kt(kt):
            w2s = w2s_pool.tile([128, H], F32R, tag="w2s")
            nc.sync.dma_start(out=w2s, in_=w2t[kt * 128:(kt + 1) * 128, :])
            for tsub in range(TSUBS):
                if kt < HT:
                    lhsT = state['mixT'][:, kt, tsub * 128:(tsub + 1) * 128]
                else:
                    lhsT = state['q_r'][:, kt - HT, tsub * 128:(tsub + 1) * 128]
                for nchk in range(2):
                    nc.tensor.matmul(
                        state['pso'][tsub * 2 + nchk],
                        lhsT,
                        w2s[:, nchk * 512:(nchk + 1) * 512],
                        start=(kt == 0), stop=(kt == KT2 - 1))

        def finish(tsub):
            ost = ostage.tile([128, H], F32, tag="ostage")
            for nchk in range(2):
                nc.vector.tensor_tensor(state['pso'][tsub * 2 + nchk],
                                        state['pso'][tsub * 2 + nchk],
                                        b2bc[:, nchk * 512:(nchk + 1) * 512],
                                        op=ALU.add)
                nc.scalar.activation(ost[:, nchk * 512:(nchk + 1) * 512],
                                     state['pso'][tsub * 2 + nchk], AF.Tanh)
            r0 = t0 + tsub * 128
            nc.sync.dma_start(out=a_out[b, r0:r0 + 128, :], in_=ost)

        prologue = [attnt_alloc]
        for tsub in range(TSUBS):
            for sg in range(4):
                prologue.append(lambda ts=tsub, s=sg: attnt_tr(ts, s))
        prologue.append(qr_build)

        rest = [mix_alloc]
        for st in range(ST):
            rest.append(lambda s=st: mix_st(s))
        for tsub in range(TSUBS):
            rest.append(lambda ts=tsub: mn_copy(ts))
        rest.append(mixt_alloc)
        for tsub in range(TSUBS):
            for hg in range(2):
                rest.append(lambda ts=tsub, h=hg: mixt_tr(ts, h))
        rest.append(lin_alloc)
        for kt in range(KT2):
            rest.append(lambda k=kt: lin_kt(k))
        for tsub in range(TSUBS):
            rest.append(lambda ts=tsub: finish(ts))
        return prologue, rest

    import itertools

    def chain(*gens):
        return itertools.chain(*[g for g in gens if g is not None])

    # prologue: weights interleaved with context(0), then A/B(0)
    wth = list(weight_thunks())
    ctt = list(ct_thunks(0))
    wi = 0
    for k, th in enumerate(ctt):
        th()
        target = (k + 1) * len(wth) // len(ctt)
        while wi < target:
            wth[wi]()
            wi += 1
    for th in ab_thunks(0):
        th()
    for idx, (b, ck) in enumerate(jobs):
        primary = [scores_thunks(idx)]
        if idx + 1 < len(jobs):
            nb, nck = jobs[idx + 1]
            primary.append(ab_thunks(idx + 1))
            if nb != b:
                primary.append(ct_thunks(nb))
        p = list(chain(*primary))
        if idx > 0:
            pro, rest = tail_thunks(idx - 1)
            for th in pro:
                th()
        else:
            rest = []
        # proportional interleave: spread `rest` across `p`
        np_, ns = len(p), len(rest)
        si = 0
        for k, th in enumerate(p):
            th()
            target = (k + 1) * ns // np_ if np_ else ns
            while si < target:
                rest[si]()
                si += 1
        while si < ns:
            rest[si]()
            si += 1
    pro, rest = tail_thunks(len(jobs) - 1)
    for th in pro + rest:
        th()


def _split_sync_waits(nc):
    """This walrus/ISA build accepts at most ONE sync-wait command per
    instruction, but Tile's sem-assigner can attach several (phase-first
    instructions, kernel-tail drain).  Split: keep the last wait on the
    instruction, hoist the rest onto same-engine NoOps inserted just before."""
    n_split = 0
    for fn in nc.m.functions:
        for blk in fn.blocks:
            insts = blk.instructions
            out = []
            changed = False
            for inst in insts:
                si = inst.sync_info
                waits = list(si.on_wait) if (si and si.on_wait) else []
                if len(waits) > 1:
                    for w in waits[:-1]:
                        nop = mybir.InstNoOp(
                            name=f"waitsplit-{nc.next_id()}",
                            engine=inst.engine,
                            sync_info=mybir.SyncInfo(on_wait=[w], on_update=[]),
                        )
                        out.append(nop)
                    inst.sync_info = mybir.SyncInfo(
                        on_wait=[waits[-1]], on_update=list(si.on_update or []))
                    n_split += 1
                    changed = True
                out.append(inst)
            if changed:
                blk.instructions = out
    return n_split


_CACHED_NC = {}


def _build(split_waits=True):
    if split_waits in _CACHED_NC:
        return _CACHED_NC[split_waits]
    nc = bass.Bass("TRN2", target_bir_lowering=False, debug=False)
    io = {
        "output": nc.dram_tensor("output", [BL, T, H], F32, kind="ExternalInput").ap(),
        "context": nc.dram_tensor("context", [BL, S, H], F32, kind="ExternalInput").ap(),
        "attn_weight": nc.dram_tensor("attn_weight", [H, H], F32, kind="ExternalInput").ap(),
        "linear_out_w": nc.dram_tensor("linear_out_w", [H, 2 * H], F32, kind="ExternalInput").ap(),
        "linear_out_b": nc.dram_tensor("linear_out_b", [H], F32, kind="ExternalInput").ap(),
        "out": nc.dram_tensor("out", [BL, T, H], F32, kind="ExternalOutput").ap(),
        "attn": nc.dram_tensor("attn", [BL, T, S], F32, kind="ExternalOutput").ap(),
    }
    with tile.TileContext(nc) as tc:
        with ExitStack() as ctx:
            _emit(ctx, tc, io)
    if split_waits:
        # CoreSim can't execute the bare NoOps; only split for the HW path.
        _split_sync_waits(nc)
    _CACHED_NC[split_waits] = nc
    return nc


def make_in_maps(inputs):
    in_maps = []
    for c in range(NCORES):
        b0 = c * BL
        in_maps.append({
            "output": np.ascontiguousarray(inputs["output"][b0:b0 + BL], dtype=np.float32),
            "context": np.ascontiguousarray(inputs["context"][b0:b0 + BL], dtype=np.float32),
            "attn_weight": np.ascontiguousarray(inputs["attn_weight"], dtype=np.float32),
            "linear_out_w": np.ascontiguousarray(inputs["linear_out_w"], dtype=np.float32),
            "linear_out_b": np.ascontiguousarray(inputs["linear_out_b"], dtype=np.float32),
        })
    return in_maps


LAST_RESULT = None


def kernel(**inputs):
    global LAST_RESULT
    nc = _build()
    in_maps = make_in_maps(inputs)
    trace = os.environ.get("KERNEL_TRACE", "0") == "1"
    res = bass_utils.run_bass_kernel_spmd(
        nc, in_maps, core_ids=list(range(NCORES)), trace=trace)
    LAST_RESULT = res
    out = np.concatenate([r["out"] for r in res.results], axis=0)
    attn = np.concatenate([r["attn"] for r in res.results], axis=0)
    return out, attn


if __name__ == "__main__":
    rng = np.random.default_rng(0)
    inputs = {
        "output": rng.standard_normal((B, T, H), dtype=np.float32),
        "context": rng.standard_normal((B, S, H), dtype=np.float32),
        "attn_weight": (rng.standard_normal((H, H)) / np.sqrt(H)).astype(np.float32),
        "linear_out_w": (rng.standard_normal((H, 2 * H)) / np.sqrt(2 * H)).astype(np.float32),
        "linear_out_b": (rng.standard_normal(H) * 0.01).astype(np.float32),
    }
    out, attn = kernel(**inputs)
    print("out", out.shape, "attn", attn.shape)


# revision 28
# speedup vs baseline: 1.4275x; 1.0313x over previous
"""Trainium2 Bass kernel for a cross-attention nn.Module.

Computes, for inputs (all fp32):
    q      = output @ attn_weight.T              [B,T,H]
    attn   = softmax(q @ context.T, axis=-1)     [B,T,S]
    mix    = attn @ context                      [B,T,H]
    out    = tanh(concat(mix, q) @ linear_out_w.T + linear_out_b)
Returns (out, attn).

Sharding: data-parallel over batch, 2 batches per NeuronCore x 8 cores,
no collectives.  Inside each core:
  - q and scores matmuls run in true fp32 (softmax is extremely sensitive:
    near-one-hot rows; bf16/tf32 scores give absmax errors ~0.02-0.16).
  - mix and linear_out matmuls run as float32r (full PE rate at N>=256).
  - layouts: contraction dims must sit on SBUF partitions, so context is
    PE-transposed once per batch (cT), attn is transposed per tile (attnT),
    mix is computed in natural [t,h] layout then transposed for linear_out.
"""

import os
import sys
import types
import numpy as np
from contextlib import ExitStack

import concourse.bass as bass
import concourse.mybir as mybir
import concourse.tile as tile
from concourse import bass_utils
from concourse.masks import make_identity


def _ensure_ntff_hook():
    """This deployment's antenv package lacks axon_hooks, which
    run_bass_kernel_spmd(trace=True) imports under axon.  Register a shim that
    drives NTFF profiling via ctypes into libaxon_pjrt.so (same contract as
    trn_agent_boot._ntff_profile_via_ctypes)."""
    try:
        from antenv.axon_hooks import get_axon_ntff_profile_hook  # noqa: F401
        return
    except ImportError:
        pass
    import contextlib
    import ctypes

    so_path = "/opt/axon/libaxon_pjrt.so"
    hook = None
    if os.path.exists(so_path):
        lib = ctypes.CDLL(so_path)
        if hasattr(lib, "axon_start_nrt_profile"):
            lib.axon_start_nrt_profile.argtypes = [
                ctypes.POINTER(ctypes.c_int64), ctypes.c_size_t]
            lib.axon_start_nrt_profile.restype = ctypes.c_int64
            lib.axon_stop_nrt_profile.argtypes = [ctypes.c_char_p]
            lib.axon_stop_nrt_profile.restype = ctypes.c_int64

            @contextlib.contextmanager
            def _hook(output_dir, device_ids):
                import jax
                jax.devices()
                if device_ids:
                    ids = (ctypes.c_int64 * len(device_ids))(*device_ids)
                    rc = lib.axon_start_nrt_profile(ids, len(device_ids))
                else:
                    rc = lib.axon_start_nrt_profile(None, 0)
                if rc != 0:
                    raise RuntimeError(f"axon_start_nrt_profile rc={rc}")
                try:
                    yield
                finally:
                    n = lib.axon_stop_nrt_profile(str(output_dir).encode())
                    print(f"ntff profile: {n} file(s) -> {output_dir}")

            hook = _hook

    mod = types.ModuleType("antenv.axon_hooks")
    mod._hook = hook
    mod.get_axon_ntff_profile_hook = lambda: mod._hook
    mod.set_axon_ntff_profile_hook = lambda h: setattr(mod, "_hook", h)
    sys.modules["antenv.axon_hooks"] = mod
    import antenv
    antenv.axon_hooks = mod


_ensure_ntff_hook()

F32 = mybir.dt.float32
F32R = mybir.dt.float32r
BF16 = mybir.dt.bfloat16
AF = mybir.ActivationFunctionType
ALU = mybir.AluOpType
AX = mybir.AxisListType

B, T, S, H = 16, 1024, 2048, 1024
NCORES = 8
BL = B // NCORES            # batches per core
T_CHK = 256                 # t rows per pipeline chunk
N_CHUNKS = T // T_CHK       # 4 per batch
TSUBS = T_CHK // 128        # 2 t-tiles per chunk
HT = H // 128               # 8 feature tiles
ST = S // 128               # 16 context-position tiles
KT2 = 2 * H // 128          # 16 contraction tiles for linear_out


def _emit(ctx: ExitStack, tc: "tile.TileContext", io: dict):
    nc = tc.nc
    a_out, a_attn = io["out"], io["attn"]
    a_output, a_context = io["output"], io["context"]
    a_w, a_w2, a_b2 = io["attn_weight"], io["linear_out_w"], io["linear_out_b"]

    const = ctx.enter_context(tc.tile_pool(name="const", bufs=1))
    ct_pool = ctx.enter_context(tc.tile_pool(name="ct", bufs=1))
    loadc = ctx.enter_context(tc.tile_pool(name="loadc", bufs=3))
    loadout = ctx.enter_context(tc.tile_pool(name="loadout", bufs=1))
    outT_pool = ctx.enter_context(tc.tile_pool(name="outT", bufs=1))
    q_pool = ctx.enter_context(tc.tile_pool(name="q", bufs=2))
    attnT_pool = ctx.enter_context(tc.tile_pool(name="attnT", bufs=1))
    mixnat_pool = ctx.enter_context(tc.tile_pool(name="mixnat", bufs=2))
    mixT_pool = ctx.enter_context(tc.tile_pool(name="mixT", bufs=1))
    astage = ctx.enter_context(tc.tile_pool(name="astage", bufs=2))
    w2s_pool = ctx.enter_context(tc.tile_pool(name="w2s", bufs=2))
    small = ctx.enter_context(tc.tile_pool(name="small", bufs=3))
    dram = ctx.enter_context(tc.tile_pool(name="dram", bufs=1, space="DRAM"))
    ps_s = ctx.enter_context(tc.tile_pool(name="ps_s", bufs=2, space="PSUM"))
    ps_long = ctx.enter_context(tc.tile_pool(name="ps_long", bufs=4, space="PSUM"))
    ps_mm = ctx.enter_context(tc.tile_pool(name="ps_mm", bufs=2, space="PSUM"))

    def transpose_to_psum(src_ap, n, ident):
        assert n <= 4
        pst_flat = ps_mm.tile([128, 512], F32, tag="mm", name="pst")
        pst = pst_flat.rearrange("p (a c) -> p a c", a=4)
        for i in range(n):
            nc.tensor.transpose(pst[:, i, :], src_ap[:, i * 128:(i + 1) * 128], ident)
        return pst

    def transpose_pack(src_ap, n, dst_ap, ident, on_act=False):
        pst = transpose_to_psum(src_ap, n, ident)
        if on_act:
            nc.scalar.copy(dst_ap, pst[:, :n, :])
        else:
            nc.vector.tensor_copy(dst_ap, pst[:, :n, :])

    def transpose_split(src_ap, n, dst_hi, dst_lo, ident):
        pst = transpose_to_psum(src_ap, n, ident)
        nc.scalar.copy(dst_hi, pst[:, :n, :])
        nc.vector.tensor_tensor(dst_lo, pst[:, :n, :], dst_hi, op=ALU.subtract)

    # ---------------- phase 0: constants ----------------
    ident = const.tile([128, 128], F32)
    make_identity(nc, ident)

    b2bc = const.tile([128, H], F32)
    nc.gpsimd.dma_start(out=b2bc, in_=a_b2.partition_broadcast(128))

    WTh = const.tile([128, HT, H], BF16)
    WTl = const.tile([128, HT, H], BF16)
    w2t = dram.tile([2 * H, H], F32R)

    def weight_thunks():
        def wt_og(og):
            wl = astage.tile([128, 2 * H], F32, tag="astage")
            nc.sync.dma_start(out=wl[:, :H], in_=a_w[og * 128:(og + 1) * 128, :])
            for hg in range(2):
                s1 = slice(hg * 4, (hg + 1) * 4)
                s2 = slice(og * 128, (og + 1) * 128)
                transpose_split(wl[:, hg * 512:(hg + 1) * 512], 4,
                                WTh[:, s1, s2], WTl[:, s1, s2], ident)

        def w2t_og(og):
            w2l = astage.tile([128, 2 * H], F32, tag="astage")
            nc.sync.dma_start(out=w2l, in_=a_w2[og * 128:(og + 1) * 128, :])
            for kg in range(4):
                stg = mixnat_pool.tile([128, 4, 128], F32R, tag="mixnat")
                pst = transpose_to_psum(w2l[:, kg * 512:(kg + 1) * 512], 4, ident)
                nc.vector.tensor_copy(stg, pst)
                dst = w2t[kg * 512:(kg + 1) * 512, og * 128:(og + 1) * 128]
                nc.sync.dma_start(out=dst.rearrange("(a p) o -> p a o", p=128), in_=stg)

        for og in range(HT):
            yield (lambda o=og: wt_og(o))
        for og in range(HT):
            yield (lambda o=og: w2t_og(o))

    # ---------------- pipelined chunk jobs ----------------
    # Per iteration j we interleave (thunk-by-thunk) the emission of:
    #   - scores+softmax of job j          (DMA-free, dense PE work)
    #   - the "tail" of job j-1            (attnT transposes, mix, mixT, lin --
    #                                       DMA-hungry, latency-bound)
    # so the PE never drains while cm/w2s streams catch up, and the DMA
    # bursts spread over the whole job span.  Then A/B (load+transpose
    # output, q matmuls) for job j+1.
    jobs = [(b, ck) for b in range(BL) for ck in range(N_CHUNKS)]
    cT = {}
    qbuf = {}
    astash = {}

    def scores_thunks(idx):
        """Dep-closed emission thunks for scores+softmax of job idx.
        sc-outer: each psum tile fully accumulates then evacuates within
        one thunk, so interleaved streams can never invert engine orders."""
        b, ck = jobs[idx]
        t0 = ck * T_CHK
        q_hi, q_lo = qbuf[idx]
        cTh, cTl = cT[b]
        state = {}

        def sc_block(tsub, sc):
            tsl = slice(tsub * 128, (tsub + 1) * 128)
            pss = ps_s.tile([128, 512], F32, tag="ps", name=f"pss{idx}_{tsub}_{sc}")
            passes = [(q_hi, cTh), (q_hi, cTl), (q_lo, cTh)]
            for kt in range(HT):
                for pi, (qsrc, csrc) in enumerate(passes):
                    nc.tensor.matmul(
                        pss,
                        qsrc[:, kt, tsl],
                        csrc[:, kt, sc * 512:(sc + 1) * 512],
                        start=(kt == 0 and pi == 0),
                        stop=(kt == HT - 1 and pi == 2))
            ast = state[tsub]
            if sc % 2 == 0:
                nc.vector.tensor_copy(ast[:, sc * 512:(sc + 1) * 512], pss)
            else:
                nc.scalar.copy(ast[:, sc * 512:(sc + 1) * 512], pss)

        def softmax(tsub):
            r0 = t0 + tsub * 128
            ast = state[tsub]
            stats = small.tile([128, 4], F32, tag="sm")
            nm, lsum, rinv = stats[:, 0:1], stats[:, 1:2], stats[:, 2:3]
            nc.vector.tensor_reduce(nm, ast, axis=AX.X, op=ALU.max, negate=True)
            nc.scalar.activation(ast, ast, AF.Exp, bias=nm, scale=1.0, accum_out=lsum)
            nc.vector.reciprocal(rinv, lsum)
            nc.vector.tensor_scalar_mul(ast, ast, rinv)
            nc.sync.dma_start(out=a_attn[b, r0:r0 + 128, :], in_=ast)
            astash.setdefault(idx, {})[tsub] = ast

        for tsub in range(TSUBS):
            def ast_alloc(ts=tsub):
                state[ts] = astage.tile([128, S], F32, tag="astage",
                                        name=f"ast{idx}_{ts}")
            yield ast_alloc
            for sc in range(4):
                yield (lambda ts=tsub, s=sc: sc_block(ts, s))
            yield (lambda ts=tsub: softmax(ts))

    def ab_thunks(idx):
        """Dep-closed thunks for output-load/transpose + q of job idx."""
        b, ck = jobs[idx]
        t0 = ck * T_CHK
        state = {}

        def alloc():
            state['oTh'] = outT_pool.tile([128, HT, T_CHK], BF16, tag="oth",
                                          name=f"oth{idx}")
            state['oTl'] = outT_pool.tile([128, HT, T_CHK], BF16, tag="otl",
                                          name=f"otl{idx}")
            state['q_hi'] = q_pool.tile([128, HT, T_CHK], BF16, tag="qhi",
                                        name=f"qhi{idx}", bufs=2)
            state['q_lo'] = q_pool.tile([128, HT, T_CHK], BF16, tag="qlo",
                                        name=f"qlo{idx}", bufs=2)
            qbuf[idx] = (state['q_hi'], state['q_lo'])

        def load_tr(tsub):
            ld = loadout.tile([128, H], F32, tag="oload")
            nc.sync.dma_start(
                out=ld, in_=a_output[b, t0 + tsub * 128:t0 + (tsub + 1) * 128, :])
            for hg in range(2):
                s1 = slice(hg * 4, (hg + 1) * 4)
                s2 = slice(tsub * 128, (tsub + 1) * 128)
                transpose_split(ld[:, hg * 512:(hg + 1) * 512], 4,
                                state['oTh'][:, s1, s2], state['oTl'][:, s1, s2],
                                ident)

        def q_ot(ot):
            psq = ps_mm.tile([128, 512], F32, tag="mm")
            tgt = psq[:, :T_CHK]
            for kt in range(HT):
                passes = [(state['oTh'], 0), (state['oTl'], 0), (state['oTh'], 1)]
                srcs = [(WTh, state['oTh']), (WTh, state['oTl']), (WTl, state['oTh'])]
                for pi, (wsrc, osrc) in enumerate(srcs):
                    nc.tensor.matmul(
                        tgt,
                        wsrc[:, kt, ot * 128:(ot + 1) * 128],
                        osrc[:, kt, :],
                        start=(kt == 0 and pi == 0),
                        stop=(kt == HT - 1 and pi == 2))
            osl = slice(ot, ot + 1)
            view = tgt.rearrange("p (a c) -> p a c", a=1)
            nc.scalar.copy(state['q_hi'][:, osl, :], view)
            nc.vector.tensor_tensor(state['q_lo'][:, osl, :], view,
                                    state['q_hi'][:, osl, :], op=ALU.subtract)

        yield alloc
        for tsub in range(TSUBS):
            yield (lambda ts=tsub: load_tr(ts))
        for ot in range(HT):
            yield (lambda o=ot: q_ot(o))

    def ct_thunks(b):
        state = {}

        def alloc():
            state['h'] = ct_pool.tile([128, HT, S], BF16, tag="cth", name=f"cth{b}")
            state['l'] = ct_pool.tile([128, HT, S], BF16, tag="ctl", name=f"ctl{b}")
            cT[b] = (state['h'], state['l'])

        def one(st):
            cl = loadc.tile([128, H], F32, tag="cload")
            nc.sync.dma_start(out=cl, in_=a_context[b, st * 128:(st + 1) * 128, :])
            for hg in range(2):
                s1, s2 = slice(hg * 4, (hg + 1) * 4), slice(st * 128, (st + 1) * 128)
                transpose_split(cl[:, hg * 512:(hg + 1) * 512], 4,
                                state['h'][:, s1, s2], state['l'][:, s1, s2], ident)

        yield alloc
        for st in range(ST):
            yield (lambda s=st: one(s))

    def tail_thunks(idx):
        """attnT/qr prologue (uninterleaved) then mix/mixT/lin thunks.
        Every pool alloc only waits on tiles whose readers were emitted
        earlier, so interleaving cannot create cross-engine wait cycles."""
        b, ck = jobs[idx]
        t0 = ck * T_CHK
        state = {}

        def attnt_alloc():
            state['attnT'] = attnT_pool.tile([128, ST, T_CHK], F32R, tag="attnT",
                                             name=f"attnT{idx}")

        def attnt_tr(tsub, sg):
            asts = astash[idx]
            tsl = slice(tsub * 128, (tsub + 1) * 128)
            transpose_pack(asts[tsub][:, sg * 512:(sg + 1) * 512], 4,
                           state['attnT'][:, sg * 4:(sg + 1) * 4, tsl], ident,
                           on_act=(sg % 2 == 1))

        def qr_build():
            q_hi, q_lo = qbuf[idx]
            qr = q_pool.tile([128, HT, T_CHK], F32R, tag="qr",
                             name=f"qr{idx}", bufs=1)
            nc.vector.tensor_tensor(qr, q_hi, q_lo, op=ALU.add)
            state['q_r'] = qr

        def mix_alloc():
            state['psm'] = [ps_long.tile([128, 512], F32, tag="ml",
                                         name=f"psm{idx}_{i}")
                            for i in range(TSUBS * 2)]

        def mix_st(st):
            cm = loadc.tile([128, H], F32R, tag="cload")
            nc.gpsimd.dma_start(out=cm, in_=a_context[b, st * 128:(st + 1) * 128, :])
            for tsub in range(TSUBS):
                lhsT = state['attnT'][:, st, tsub * 128:(tsub + 1) * 128]
                for nchk in range(2):
                    nc.tensor.matmul(
                        state['psm'][tsub * 2 + nchk],
                        lhsT,
                        cm[:, nchk * 512:(nchk + 1) * 512],
                        start=(st == 0), stop=(st == ST - 1))

        def mn_copy(tsub):
            mn = mixnat_pool.tile([128, H], F32, tag="mixnat",
                                  name=f"mn{idx}_{tsub}")
            nc.vector.tensor_copy(mn[:, :512], state['psm'][tsub * 2])
            nc.scalar.copy(mn[:, 512:], state['psm'][tsub * 2 + 1])
            state.setdefault('mns', {})[tsub] = mn

        def mixt_alloc():
            state['mixT'] = mixT_pool.tile([128, HT, T_CHK], F32R, tag="mixT",
                                           name=f"mixT{idx}")

        def mixt_tr(tsub, hg):
            transpose_pack(state['mns'][tsub][:, hg * 512:(hg + 1) * 512], 4,
                           state['mixT'][:, hg * 4:(hg + 1) * 4,
                                         tsub * 128:(tsub + 1) * 128], ident,
                           on_act=(hg == 1))

        def lin_alloc():
            state['pso'] = [ps_long.tile([128, 512], F32, tag="ml",
                                         name=f"pso{idx}_{i}")
                            for i in range(TSUBS * 2)]

        def lin_kt(kt):
            w2s = w2s_pool.tile([128, H], F32R, tag="w2s")
            nc.sync.dma_start(out=w2s, in_=w2t[kt * 128:(kt + 1) * 128, :])
            for tsub in range(TSUBS):
                if kt < HT:
                    lhsT = state['mixT'][:, kt, tsub * 128:(tsub + 1) * 128]
                else:
                    lhsT = state['q_r'][:, kt - HT, tsub * 128:(tsub + 1) * 128]
                for nchk in range(2):
                    nc.tensor.matmul(
                        state['pso'][tsub * 2 + nchk],
                        lhsT,
                        w2s[:, nchk * 512:(nchk + 1) * 512],
                        start=(kt == 0), stop=(kt == KT2 - 1))

        def finish(tsub):
            ost = mixnat_pool.tile([128, H], F32, tag="mixnat")
            for nchk in range(2):
                nc.vector.tensor_tensor(state['pso'][tsub * 2 + nchk],
                                        state['pso'][tsub * 2 + nchk],
                                        b2bc[:, nchk * 512:(nchk + 1) * 512],
                                        op=ALU.add)
                nc.scalar.activation(ost[:, nchk * 512:(nchk + 1) * 512],
                                     state['pso'][tsub * 2 + nchk], AF.Tanh)
            r0 = t0 + tsub * 128
            nc.sync.dma_start(out=a_out[b, r0:r0 + 128, :], in_=ost)

        prologue = [attnt_alloc]
        for tsub in range(TSUBS):
            for sg in range(4):
                prologue.append(lambda ts=tsub, s=sg: attnt_tr(ts, s))
        prologue.append(qr_build)

        rest = [mix_alloc]
        for st in range(ST):
            rest.append(lambda s=st: mix_st(s))
        for tsub in range(TSUBS):
            rest.append(lambda ts=tsub: mn_copy(ts))
        rest.append(mixt_alloc)
        for tsub in range(TSUBS):
            for hg in range(2):
                rest.append(lambda ts=tsub, h=hg: mixt_tr(ts, h))
        rest.append(lin_alloc)
        for kt in range(KT2):
            rest.append(lambda k=kt: lin_kt(k))
        for tsub in range(TSUBS):
            rest.append(lambda ts=tsub: finish(ts))
        return prologue, rest

    import itertools

    def chain(*gens):
        return itertools.chain(*[g for g in gens if g is not None])

    # prologue: weights interleaved with context(0), then A/B(0)
    wth = list(weight_thunks())
    ctt = list(ct_thunks(0))
    wi = 0
    for k, th in enumerate(ctt):
        th()
        target = (k + 1) * len(wth) // len(ctt)
        while wi < target:
            wth[wi]()
            wi += 1
    for th in ab_thunks(0):
        th()
    for idx, (b, ck) in enumerate(jobs):
        primary = [scores_thunks(idx)]
        if idx + 1 < len(jobs):
            nb, nck = jobs[idx + 1]
            primary.append(ab_thunks(idx + 1))
            if nb != b:
                primary.append(ct_thunks(nb))
        p = list(chain(*primary))
        if idx > 0:
            pro, rest = tail_thunks(idx - 1)
            for th in pro:
                th()
        else:
            rest = []
        # proportional interleave: spread `rest` across `p`
        np_, ns = len(p), len(rest)
        si = 0
        for k, th in enumerate(p):
            th()
            target = (k + 1) * ns // np_ if np_ else ns
            while si < target:
                rest[si]()
                si += 1
        while si < ns:
            rest[si]()
            si += 1
    pro, rest = tail_thunks(len(jobs) - 1)
    for th in pro + rest:
        th()


def _split_sync_waits(nc):
    """This walrus/ISA build accepts at most ONE sync-wait command per
    instruction, but Tile's sem-assigner can attach several (phase-first
    instructions, kernel-tail drain).  Split: keep the last wait on the
    instruction, hoist the rest onto same-engine NoOps inserted just before."""
    n_split = 0
    for fn in nc.m.functions:
        for blk in fn.blocks:
            insts = blk.instructions
            out = []
            changed = False
            for inst in insts:
                si = inst.sync_info
                waits = list(si.on_wait) if (si and si.on_wait) else []
                if len(waits) > 1:
                    for w in waits[:-1]:
                        nop = mybir.InstNoOp(
                            name=f"waitsplit-{nc.next_id()}",
                            engine=inst.engine,
                            sync_info=mybir.SyncInfo(on_wait=[w], on_update=[]),
                        )
                        out.append(nop)
                    inst.sync_info = mybir.SyncInfo(
                        on_wait=[waits[-1]], on_update=list(si.on_update or []))
                    n_split += 1
                    changed = True
                out.append(inst)
            if changed:
                blk.instructions = out
    return n_split


_CACHED_NC = {}


def _build(split_waits=True):
    if split_waits in _CACHED_NC:
        return _CACHED_NC[split_waits]
    nc = bass.Bass("TRN2", target_bir_lowering=False, debug=False)
    io = {
        "output": nc.dram_tensor("output", [BL, T, H], F32, kind="ExternalInput").ap(),
        "context": nc.dram_tensor("context", [BL, S, H], F32, kind="ExternalInput").ap(),
        "attn_weight": nc.dram_tensor("attn_weight", [H, H], F32, kind="ExternalInput").ap(),
        "linear_out_w": nc.dram_tensor("linear_out_w", [H, 2 * H], F32, kind="ExternalInput").ap(),
        "linear_out_b": nc.dram_tensor("linear_out_b", [H], F32, kind="ExternalInput").ap(),
        "out": nc.dram_tensor("out", [BL, T, H], F32, kind="ExternalOutput").ap(),
        "attn": nc.dram_tensor("attn", [BL, T, S], F32, kind="ExternalOutput").ap(),
    }
    with tile.TileContext(nc) as tc:
        with ExitStack() as ctx:
            _emit(ctx, tc, io)
    if split_waits:
        # CoreSim can't execute the bare NoOps; only split for the HW path.
        _split_sync_waits(nc)
    _CACHED_NC[split_waits] = nc
    return nc


def make_in_maps(inputs):
    in_maps = []
    for c in range(NCORES):
        b0 = c * BL
        in_maps.append({
            "output": np.ascontiguousarray(inputs["output"][b0:b0 + BL], dtype=np.float32),
            "context": np.ascontiguousarray(inputs["context"][b0:b0 + BL], dtype=np.float32),
            "attn_weight": np.ascontiguousarray(inputs["attn_weight"], dtype=np.float32),
            "linear_out_w": np.ascontiguousarray(inputs["linear_out_w"], dtype=np.float32),
            "linear_out_b": np.ascontiguousarray(inputs["linear_out_b"], dtype=np.float32),
        })
    return in_maps


LAST_RESULT = None


def kernel(**inputs):
    global LAST_RESULT
    nc = _build()
    in_maps = make_in_maps(inputs)
    trace = os.environ.get("KERNEL_TRACE", "0") == "1"
    res = bass_utils.run_bass_kernel_spmd(
        nc, in_maps, core_ids=list(range(NCORES)), trace=trace)
    LAST_RESULT = res
    out = np.concatenate([r["out"] for r in res.results], axis=0)
    attn = np.concatenate([r["attn"] for r in res.results], axis=0)
    return out, attn


if __name__ == "__main__":
    rng = np.random.default_rng(0)
    inputs = {
        "output": rng.standard_normal((B, T, H), dtype=np.float32),
        "context": rng.standard_normal((B, S, H), dtype=np.float32),
        "attn_weight": (rng.standard_normal((H, H)) / np.sqrt(H)).astype(np.float32),
        "linear_out_w": (rng.standard_normal((H, 2 * H)) / np.sqrt(2 * H)).astype(np.float32),
        "linear_out_b": (rng.standard_normal(H) * 0.01).astype(np.float32),
    }
    out, attn = kernel(**inputs)
    print("out", out.shape, "attn", attn.shape)
